# revision 26
# baseline (speedup 1.0000x reference)
"""Trainium2 Bass kernel for nn_EquivariantBiLinear.

Math (per batch row b):
    Y[k, b] = sum_nu W_g[mu, nu] * x[b, bid_g[nu*r+rho]]   (k = off_g + mu*r + rho)
    out[b, o] = 0.1 * sum_i Y[W_invperm[o*256+i], b] * x[b, i]

Sharding: 4-way over batch x 2-way over k-space (8 cores; host adds the
two k-partials per batch slice). Per core: 256 chunks of 128 k-rows x
512 batch cols. Per chunk: group GEMM (fp16, N=512) -> PSUM, DVE mult
by the host-gathered xg = x[b, i_k]/4 -> z (fp16), then a one-hot fp8
scatter matmul accumulates z into the persistent output PSUM bank(s).

Key trick vs the naive layout: the scatter previously needed 2 matmuls
per chunk (its 128 target o-rows span both 128-row PSUM banks). Since
the GEMM's mu-tiling within each (group, rho)-stream is free, the host
sorts each stream's k's by target o-half so almost every chunk is
*pure* (single-bank scatter: 1 matmul). Boundary/parity chunks stay
"mixed" (2 matmuls). All weights are streamed per-chunk in the sorted
order (pure layout transform on the host). The two k-cores must share
one SPMD program, so per (stream, class) chunk counts are made even by
demoting a few pure chunks to mixed; both cores then run an identical
slot sequence with different streamed data. Scatter matmuls drop from
512 to ~308 per core.

Scheduling: slots are ordered big(g0/g1)-small(g2/g3) alternating so
every PSUM-pair step carries enough PE work to hide the PSUM->SBUF
copy; xrep is loaded as 60 per-(group,rho) blocks posted in first-use
order so the first GEMM starts ~3us in; DMA fetches are batched into
>=1KB/partition slabs to cut sequencer post overhead. The o-half-0
output bank finishes before the pure-1 phase, so its epilogue overlaps
the remaining chunks.
"""

import sys

if "/opt/trn_rl_repo" not in sys.path:
    sys.path.insert(0, "/opt/trn_rl_repo")

from contextlib import ExitStack

import numpy as np

import concourse.bacc as bacc
import concourse.mybir as mybir
import concourse.tile as tile
from concourse.bass_utils import run_bass_kernel_spmd
from concourse.masks import make_identity

GROUPS = [(512, 1, 16384), (256, 4, 4096), (128, 16, 1024), (64, 64, 256)]
OFFS = [0, 16384, 32768, 49152]
X = 256
B = 2048
NCORES = 8
BS = 512  # batch rows per core
NSLOT = 256  # chunks per core

F32 = mybir.dt.float32
FP16 = mybir.dt.float16
FP8 = mybir.dt.float8e4

# earliest slot index per group (xrep warm-up)
GSTART = [8, 0, 20, 40]


def _streams(pairs3):
    """List of (g, stream_id, k-array in canonical order)."""
    out = []
    for gi, (n, r, m) in enumerate(GROUPS):
        off = OFFS[gi]
        if gi < 3:
            for rho in range(r):
                out.append((gi, rho, off + np.arange(m) * r + rho))
        else:
            for j, (sa, sb) in enumerate(pairs3):
                out.append(
                    (
                        gi,
                        j,
                        np.concatenate(
                            [
                                off + np.arange(m) * r + sa,
                                off + np.arange(m) * r + sb,
                            ]
                        ),
                    )
                )
    return out


def _pair_g3(perm):
    """Pair g3 rho-streams so each pair's o-half-0 count sums to exactly
    256 where possible: such pairs tile into 2+2 pure chunks with no
    mixed boundary and no parity demotion."""
    H = perm // (X * X // 2)
    off = OFFS[3]
    n0 = [
        int((H[off + np.arange(256) * 64 + s] == 0).sum()) for s in range(64)
    ]
    by_val = {}
    for s, v in enumerate(n0):
        by_val.setdefault(v, []).append(s)
    pairs = []
    used = [False] * 64
    for s in range(64):
        if used[s]:
            continue
        want = 256 - n0[s]
        cand = [t for t in by_val.get(want, []) if not used[t] and t != s]
        if cand:
            t = cand[0]
            used[s] = used[t] = True
            pairs.append((s, t))
    rest = [s for s in range(64) if not used[s]]
    rest.sort(key=lambda s: n0[s])
    while rest:
        pairs.append((rest.pop(0), rest.pop(-1)))
    assert len(pairs) == 32
    return pairs


def _order_slots(per_group):
    """Merge the 4 per-group item lists into one slot order: big (g0/g1)
    and small (g2/g3) alternate so PSUM pairs carry steady PE work;
    within each size class groups interleave proportionally; GSTART
    delays a group until its xrep blocks can be resident."""

    def mk_queue(groups):
        lists = {g: list(per_group[g]) for g in groups}
        tot = {g: max(1, len(lists[g])) for g in groups}
        idx = {g: 0 for g in groups}

        def pop(slot_i, force=False):
            best, bestv = None, -1.0
            for g in groups:
                if idx[g] >= len(lists[g]):
                    continue
                if slot_i < GSTART[g] and not force:
                    continue
                v = (len(lists[g]) - idx[g]) / tot[g]
                if v > bestv:
                    best, bestv = g, v
            if best is None:
                return None
            it = lists[best][idx[best]]
            idx[best] += 1
            return it

        return pop

    nb = len(per_group[0]) + len(per_group[1])
    ns = len(per_group[2]) + len(per_group[3])
    big = mk_queue([1, 0])
    small = mk_queue([2, 3])
    out = []
    want_big = False
    for _ in range(nb + ns):
        want_big = not want_big
        it = (big if want_big else small)(len(out))
        if it is None:
            it = (small if want_big else big)(len(out))
        if it is None:
            it = big(len(out), force=True) or small(len(out), force=True)
        out.append(it)
    return out


def _plan(perm):
    """Chunk the k-space into o-half-sorted 128-tiles and build the
    shared slot schedule."""
    H = perm // (X * X // 2)
    pairs3 = _pair_g3(perm)

    chunks = {}  # (g, s, cls) -> list of klist arrays
    for gi, s, ks in _streams(pairs3):
        hh = H[ks]
        order = np.argsort(hh, kind="stable")
        ks = ks[order]
        n0 = int((hh == 0).sum())
        a0, c = divmod(n0, 128)
        mx = 1 if c else 0
        a1 = (len(ks) - n0) // 128
        guard = 0
        while (a0 % 2) or (mx % 2) or (a1 % 2):
            if a0 % 2:
                a0 -= 1
                mx += 1
            elif a1 % 2:
                a1 -= 1
                mx += 1
            guard += 1
            assert guard < 8
        tiles = [ks[t * 128 : (t + 1) * 128] for t in range(len(ks) // 128)]
        chunks[(gi, s, 0)] = tiles[:a0]
        chunks[(gi, s, 1)] = tiles[a0 : a0 + mx]
        chunks[(gi, s, 2)] = tiles[a0 + mx :]

    chunk_of = [{}, {}]
    for key, lst in chunks.items():
        assert len(lst) % 2 == 0, (key, len(lst))
        chunk_of[0][key] = lst[0::2]
        chunk_of[1][key] = lst[1::2]

    per_group = {ph: {g: [] for g in range(4)} for ph in range(2)}
    for gi, s, _ks in _streams(pairs3):
        for cls in (0, 1, 2):
            nn = len(chunk_of[0][(gi, s, cls)])
            ph = 0 if cls < 2 else 1
            per_group[ph][gi].extend([(gi, s, cls)] * nn)

    # Blend bank0-touching (cls 0/1) and pure-1 slots into one stream so
    # the DMA-heavy mix is uniform (the old A-then-B split made the A
    # phase locally DMA-bound). Front-load bank0 work to finish by ~94%
    # of the timeline so its epilogue still overlaps remaining slots.
    merged = {}
    for g in range(4):
        A, Bl = per_group[0][g], per_group[1][g]
        a, b = len(A), len(Bl)
        n = a + b
        lst = []
        ai = bi = 0
        for i in range(n):
            take_a = ai < a and (
                bi >= b or ai / max(1, a) <= i / (0.94 * max(1, n))
            )
            if take_a:
                lst.append(A[ai])
                ai += 1
            else:
                lst.append(Bl[bi])
                bi += 1
        merged[g] = lst
    slots = _order_slots(merged)
    assert len(slots) == NSLOT

    meta = []
    i0 = i1 = i23 = 0
    icol = 0
    for si, (gi, s, cls) in enumerate(slots):
        if gi == 0:
            wt = ("wt0", i0 * 512)
            i0 += 1
        elif gi == 1:
            wt = ("wt1", i1 * 256)
            i1 += 1
        else:
            wt = ("wt23", i23 * 128)
            i23 += 1
        banks = [0] if cls == 0 else ([1] if cls == 2 else [0, 1])
        cols = {}
        for bk in banks:
            cols[bk] = icol
            icol += 1
        meta.append(dict(g=gi, s=s, cls=cls, wt=wt, banks=banks, icol=cols))
    nidx = (icol + 15) // 16 * 16

    first = {0: None, 1: None}
    last = {0: None, 1: None}
    for si, m in enumerate(meta):
        for bk in m["banks"]:
            if first[bk] is None:
                first[bk] = si
            last[bk] = si
    for si, m in enumerate(meta):
        m["start"] = {bk: si == first[bk] for bk in m["banks"]}
        m["stop"] = {bk: si == last[bk] for bk in m["banks"]}
    ep0_after = last[0]

    counts = (i0, i1, i23)
    return slots, chunk_of, meta, nidx, ep0_after, counts, pairs3


def _host_prep(W0, W1, W2, W3, bid0, bid1, bid2, bid3, W_invperm):
    """Pure layout transforms of weights/indices (no arithmetic on data)."""
    import ml_dtypes

    Ws = [np.asarray(W) for W in (W0, W1, W2, W3)]
    bids = [np.asarray(b).astype(np.int64) for b in (bid0, bid1, bid2, bid3)]
    ivp = np.asarray(W_invperm).astype(np.int64)
    perm = np.empty(X * X, np.int64)
    perm[ivp] = np.arange(X * X)

    slots, chunk_of, meta, nidx, ep0_after, counts, pairs3 = _plan(perm)

    wt = [
        np.ascontiguousarray(W.reshape(m, n).T.astype(np.float16))
        for (n, r, m), W in zip(GROUPS, Ws)
    ]

    pr = np.arange(128)
    cores = []
    for kc in range(2):
        qidx = {key: 0 for key in chunk_of[kc]}
        wt0p = np.zeros((128, counts[0] * 512), np.float16)
        wt1p = np.zeros((128, counts[1] * 256), np.float16)
        wt23p = np.zeros((128, counts[2] * 128), np.float16)
        sidx = np.full((128, nidx), -1.0, np.float32)
        iks = np.empty(NSLOT * 128, np.int64)
        for si, m in enumerate(meta):
            gi, s, cls = slots[si]
            ks = chunk_of[kc][(gi, s, cls)][qidx[(gi, s, cls)]]
            qidx[(gi, s, cls)] += 1
            off = OFFS[gi]
            n, r, _m = GROUPS[gi]
            kind, woff = m["wt"]
            if gi == 3:
                mu = (ks - off) // r
                rho = (ks - off) % r
                sub = np.where(rho == pairs3[s][0], 0, 1)
                assert np.all((rho == pairs3[s][0]) | (rho == pairs3[s][1]))
                blk = np.zeros((128, 128), np.float16)
                rows = sub[None, :] * 64 + np.arange(64)[:, None]  # (64, 128)
                blk[rows, np.arange(128)[None, :]] = wt[3][:, mu]
                wt23p[:, woff : woff + 128] = blk
            else:
                mu = (ks - off) // r
                cols = wt[gi][:, mu]  # (n, 128)
                if gi == 0:
                    for kcc in range(4):
                        wt0p[:, woff + kcc * 128 : woff + (kcc + 1) * 128] = cols[
                            kcc * 128 : (kcc + 1) * 128, :
                        ]
                elif gi == 1:
                    for kcc in range(2):
                        wt1p[:, woff + kcc * 128 : woff + (kcc + 1) * 128] = cols[
                            kcc * 128 : (kcc + 1) * 128, :
                        ]
                else:
                    wt23p[:, woff : woff + 128] = cols
            pk = perm[ks]
            iks[si * 128 : (si + 1) * 128] = pk % X
            ok = pk // X
            for bk, col in m["icol"].items():
                sel = (ok // 128) == bk
                sidx[pr[sel], col] = (ok % 128)[sel]
        cores.append(dict(wt0=wt0p, wt1=wt1p, wt23=wt23p, sidx=sidx, iks=iks))

    # x-gather column map for xrep (identical to reference layout)
    colsl = []
    b0 = bids[0]
    for kcc in range(4):
        colsl.append(b0[kcc * 128 : (kcc + 1) * 128])
    b1 = bids[1].reshape(256, 4)
    for rho in range(4):
        for kcc in range(2):
            colsl.append(b1[kcc * 128 : (kcc + 1) * 128, rho])
    b2 = bids[2].reshape(128, 16)
    for rho in range(16):
        colsl.append(b2[:, rho])
    b3 = bids[3].reshape(64, 64)
    for sa, sb in pairs3:
        colsl.append(b3[pr % 64, np.where(pr // 64 == 0, sa, sb)])
    xgidx = np.ascontiguousarray(np.stack(colsl, axis=1).astype(np.int64))

    plan_key = (
        tuple(tuple(p) for p in pairs3),
        tuple(slots),
        tuple(tuple(sorted(m["icol"].items())) for m in meta),
        nidx,
        ep0_after,
        counts,
    )
    plan = dict(
        slots=slots, meta=meta, nidx=nidx, ep0_after=ep0_after, counts=counts,
        key=hash(repr(plan_key)),
    )
    return plan, cores, xgidx


def _xrep_block(gi, s, kcc=0):
    """xrep plane block index for (group, stream[, kc])."""
    if gi == 0:
        return kcc
    if gi == 1:
        return 4 + s * 2 + kcc
    if gi == 2:
        return 12 + s
    return 28 + s


def _build_nc(plan):
    meta = plan["meta"]
    nidx = plan["nidx"]
    ep0_after = plan["ep0_after"]
    n0s, n1s, n23s = plan["counts"]

    nc = bacc.Bacc("TRN2", target_bir_lowering=False, debug=False, num_devices=NCORES)

    xrep_d = nc.dram_tensor("xrep", [128, 60 * BS], FP16, kind="ExternalInput")
    xg_d = nc.dram_tensor("xg", [128, NSLOT * BS], FP16, kind="ExternalInput")
    wt0_d = nc.dram_tensor("wt0", [128, n0s * 512], FP16, kind="ExternalInput")
    wt1_d = nc.dram_tensor("wt1", [128, n1s * 256], FP16, kind="ExternalInput")
    wt23_d = nc.dram_tensor("wt23", [128, n23s * 128], FP16, kind="ExternalInput")
    sidx_d = nc.dram_tensor("sidx", [128, nidx], F32, kind="ExternalInput")
    out_d = nc.dram_tensor("out", [BS, X], F32, kind="ExternalOutput")

    # ---- fetch schedule: batched slabs, attached to the pair that is
    # PF pairs ahead of first use (negative -> preamble) ----
    PF = 6
    npair = NSLOT // 2
    fetch = [[] for _ in range(npair)]
    pre = []  # preamble fetches, in priority order

    def sched(first_use_pair, op, rank=3):
        p = first_use_pair - PF
        if p < 0:
            pre.append(((first_use_pair, rank), op))
        else:
            fetch[p].append(op)

    # xrep fetch units: g0 one [128,4BS]; g1 per-stream [128,2BS];
    # g2 stream-pairs [128,2BS]; g3 stream-quads [128,4BS]
    xrep_first = {}
    for si, m in enumerate(meta):
        gi, s = m["g"], m["s"]
        if gi == 0:
            unit = ("xr0", 0)
        elif gi == 1:
            unit = ("xr1", s)
        elif gi == 2:
            unit = ("xr2", s // 2)
        else:
            unit = ("xr3", s // 4)
        if unit not in xrep_first:
            xrep_first[unit] = (si // 2, si)
    for unit, (fu, _fs) in sorted(xrep_first.items(), key=lambda kv: kv[1]):
        sched(fu, unit, rank=1)

    # xg slabs of 4 pairs (rank demoted: only needed by the pair's mult,
    # so the startup-critical wt0/xr0 transfers go first)
    for slab in range((npair + 3) // 4):
        sched(slab * 4, ("xg", slab), rank=6)
    # wt windows: wt0 [4 g0-slots], wt1 [4 g1-slots], wt23 [16 slots]
    seen = set()
    for si, m in enumerate(meta):
        kind, woff = m["wt"]
        w = woff // 2048 if kind != "wt1" else woff // 1024
        if (kind, w) not in seen:
            seen.add((kind, w))
            sched(si // 2, (kind, w), rank=0 if kind == "wt1" else 2)
    sched(0, ("sidx", 0), rank=7)
    pre.sort(key=lambda kv: kv[0])

    with tile.TileContext(nc) as tc, ExitStack() as ctx:
        const = ctx.enter_context(tc.tile_pool(name="const", bufs=1))
        w0pool = ctx.enter_context(tc.tile_pool(name="w0pool", bufs=4))
        w1pool = ctx.enter_context(tc.tile_pool(name="w1pool", bufs=4))
        w23pool = ctx.enter_context(tc.tile_pool(name="w23pool", bufs=4))
        xgpool = ctx.enter_context(tc.tile_pool(name="xgpool", bufs=4))
        ohpool = ctx.enter_context(tc.tile_pool(name="ohpool", bufs=16))
        ypool = ctx.enter_context(tc.tile_pool(name="ypool", bufs=4))
        zpool = ctx.enter_context(tc.tile_pool(name="zpool", bufs=8))
        pgemm = ctx.enter_context(tc.tile_pool(name="pgemm", bufs=3, space="PSUM"))
        pout = ctx.enter_context(tc.tile_pool(name="pout", bufs=1, space="PSUM"))

        ident = const.tile([128, 128], F32)
        make_identity(nc, ident[:])
        iota_t = const.tile([128, 128], FP16, name="iota")
        nc.gpsimd.iota(
            iota_t[:],
            pattern=[[1, 128]],
            base=0,
            channel_multiplier=0,
            allow_small_or_imprecise_dtypes=True,
        )
        sidxt = const.tile([128, nidx], F32, name="sidxt")

        xr0t = const.tile([128, 4 * BS], FP16, name="xr0t")
        xr1t = [const.tile([128, 2 * BS], FP16, name=f"xr1_{s}") for s in range(4)]
        xr2t = [const.tile([128, 2 * BS], FP16, name=f"xr2_{j}") for j in range(8)]
        xr3t = [const.tile([128, 4 * BS], FP16, name=f"xr3_{j}") for j in range(8)]

        # All steady-state posts go to the sync sequencer: descriptors of
        # any one post spread across all 16 DMA engines, so a single ring
        # sustains full bandwidth; posting from Act would head-of-line
        # block the PSUM->SBUF copies behind pool-buffer semaphore waits.
        rb = {"n": 0, "startup": True}

        def ring(nbytes):
            if rb["startup"]:
                rb["n"] += 1
                return nc.scalar if rb["n"] % 2 else nc.sync
            return nc.sync

        state = {
            "xgq": [], "w0q": [], "w1q": [], "w23q": [],
            "xg_cur": None, "xg_off": 0,
            "w0_cur": None, "w0_off": 4, "w1_cur": None, "w1_off": 4,
            "w23_cur": None, "w23_off": 16,
            "pend": [], "ps": None,
        }

        def do_op(op):
            kind, arg = op
            if kind == "xr0":
                ring(4 * BS * 256).dma_start(xr0t[:], xrep_d[:, 0 : 4 * BS])
            elif kind == "xr1":
                lo = (4 + arg * 2) * BS
                ring(2 * BS * 256).dma_start(
                    xr1t[arg][:], xrep_d[:, lo : lo + 2 * BS]
                )
            elif kind == "xr2":
                lo = (12 + arg * 2) * BS
                ring(2 * BS * 256).dma_start(
                    xr2t[arg][:], xrep_d[:, lo : lo + 2 * BS]
                )
            elif kind == "xr3":
                lo = (28 + arg * 4) * BS
                ring(4 * BS * 256).dma_start(
                    xr3t[arg][:], xrep_d[:, lo : lo + 4 * BS]
                )
            elif kind == "sidx":
                ring(nidx * 512).dma_start(sidxt[:], sidx_d[:])
            elif kind == "xg":
                t = xgpool.tile([128, 4096], FP16, tag="xgt", name="xgt")
                lo = arg * 4096
                hi = min(lo + 4096, NSLOT * BS)
                ring((hi - lo) * 256).dma_start(t[:, : hi - lo], xg_d[:, lo:hi])
                state["xgq"].append(t)
            elif kind == "wt0":
                t = w0pool.tile([128, 2048], FP16, tag="w0t", name="w0t")
                lo = arg * 2048
                hi = min(lo + 2048, n0s * 512)
                ring((hi - lo) * 256).dma_start(t[:, : hi - lo], wt0_d[:, lo:hi])
                state["w0q"].append(t)
            elif kind == "wt1":
                t = w1pool.tile([128, 1024], FP16, tag="w1t", name="w1t")
                lo = arg * 1024
                hi = min(lo + 1024, n1s * 256)
                ring((hi - lo) * 256).dma_start(t[:, : hi - lo], wt1_d[:, lo:hi])
                state["w1q"].append(t)
            elif kind == "wt23":
                t = w23pool.tile([128, 2048], FP16, tag="w23t", name="w23t")
                lo = arg * 2048
                hi = min(lo + 2048, n23s * 128)
                ring((hi - lo) * 256).dma_start(t[:, : hi - lo], wt23_d[:, lo:hi])
                state["w23q"].append(t)

        outT_ps = [
            pout.tile([128, BS], F32, tag=f"pout{h}", name=f"pout{h}") for h in range(2)
        ]

        def flush_pending():
            for si, ohts, zv in state["pend"]:
                m = meta[si]
                for bk, lhs in ohts:
                    nc.tensor.matmul(
                        outT_ps[bk][:],
                        lhs,
                        zv,
                        start=m["start"][bk],
                        stop=m["stop"][bk],
                        skip_group_check=True,
                    )
            state["pend"].clear()

        def epilogue(hb):
            # scale out of the bank, transpose back INTO the drained bank
            # (no pgemm-pool churn), stage once to SBUF, then DMA out.
            outT_sb = zpool.tile([128, BS], F32, tag="outT_sb", name="outT_sb", bufs=2)
            nc.scalar.mul(outT_sb[:], outT_ps[hb][:], 0.4)
            for bh in range(4):
                nc.tensor.transpose(
                    outT_ps[hb][:, bh * 128 : (bh + 1) * 128],
                    outT_sb[:, bh * 128 : (bh + 1) * 128],
                    ident[:],
                )
            stg = zpool.tile([128, BS], F32, tag="outstg", name="outstg", bufs=2)
            nc.scalar.copy(stg[:], outT_ps[hb][:])
            for bh in range(4):
                nc.sync.dma_start(
                    out_d[bh * 128 : (bh + 1) * 128, hb * 128 : (hb + 1) * 128],
                    stg[:, bh * 128 : (bh + 1) * 128],
                )

        # ---- preamble fetches (startup-critical, ring-alternating) ----
        for _key, op in pre:
            do_op(op)
        rb["startup"] = False

        # ---- main loop over slots ----
        for si, m in enumerate(meta):
            gi = m["g"]
            s = m["s"]
            if si % 2 == 0:
                for op in fetch[si // 2]:
                    do_op(op)
                ps = pgemm.tile([128, 1024], F32, tag="pg", name="ps")
                state["ps"] = ps
            ps_half = state["ps"][:, (si % 2) * 512 : (si % 2 + 1) * 512]

            # GEMM
            if gi == 0:
                if state["w0_off"] == 4:
                    state["w0_cur"] = state["w0q"].pop(0)
                    state["w0_off"] = 0
                w = state["w0_cur"]
                o = state["w0_off"] * 512
                state["w0_off"] += 1
                for kcc in range(4):
                    nc.tensor.matmul(
                        ps_half,
                        w[:, o + kcc * 128 : o + (kcc + 1) * 128],
                        xr0t[:, kcc * BS : (kcc + 1) * BS],
                        start=(kcc == 0),
                        stop=(kcc == 3),
                    )
            elif gi == 1:
                if state["w1_off"] == 4:
                    state["w1_cur"] = state["w1q"].pop(0)
                    state["w1_off"] = 0
                w = state["w1_cur"]
                o = state["w1_off"] * 256
                state["w1_off"] += 1
                for kcc in range(2):
                    nc.tensor.matmul(
                        ps_half,
                        w[:, o + kcc * 128 : o + (kcc + 1) * 128],
                        xr1t[s][:, kcc * BS : (kcc + 1) * BS],
                        start=(kcc == 0),
                        stop=(kcc == 1),
                    )
            else:
                if state["w23_off"] == 16:
                    state["w23_cur"] = state["w23q"].pop(0)
                    state["w23_off"] = 0
                w = state["w23_cur"]
                o = state["w23_off"] * 128
                state["w23_off"] += 1
                rhs = (
                    xr2t[s // 2][:, (s % 2) * BS : (s % 2 + 1) * BS]
                    if gi == 2
                    else xr3t[s // 4][:, (s % 4) * BS : (s % 4 + 1) * BS]
                )
                nc.tensor.matmul(
                    ps_half,
                    w[:, o : o + 128],
                    rhs,
                    start=True,
                    stop=True,
                )

            if si % 2 == 1:
                thresh = 2 if si >= NSLOT - 16 else 6
                if len(state["pend"]) >= thresh:
                    flush_pending()
                pair = si // 2
                if state["xg_off"] == 4 or state["xg_cur"] is None:
                    state["xg_cur"] = state["xgq"].pop(0)
                    state["xg_off"] = 0
                xgt = state["xg_cur"]
                xo = state["xg_off"] * 1024
                state["xg_off"] += 1
                z16 = zpool.tile([128, 1024], FP16, tag="z16", name="z16")
                if pair >= NSLOT // 2 - 5:
                    # tail pairs: fused PSUM-read multiply shortens the
                    # end-of-stream copy->mult->scatter chain
                    nc.vector.tensor_mul(
                        z16[:], state["ps"][:], xgt[:, xo : xo + 1024]
                    )
                else:
                    yt = ypool.tile([128, 1024], FP16, tag="yt", name="yt")
                    if pair < 6:
                        nc.vector.tensor_copy(yt[:], state["ps"][:])
                    else:
                        nc.scalar.copy(yt[:], state["ps"][:])
                    nc.vector.tensor_mul(z16[:], yt[:], xgt[:, xo : xo + 1024])
                for j in (si - 1, si):
                    mm = meta[j]
                    ohts = []
                    for bk in mm["banks"]:
                        col = mm["icol"][bk]
                        ohg = ohpool.tile([128, 128], FP16, tag="ohg", name="ohg")
                        nc.vector.tensor_scalar(
                            ohg[:],
                            iota_t[:],
                            sidxt[:, col : col + 1],
                            None,
                            mybir.AluOpType.is_equal,
                        )
                        ohts.append((bk, ohg[:]))
                    zv = z16[:, (j % 2) * 512 : (j % 2 + 1) * 512]
                    state["pend"].append((j, ohts, zv))
                if si == ep0_after or si - 1 == ep0_after:
                    flush_pending()
                    epilogue(0)

        flush_pending()
        epilogue(1)

    nc.compile()
    return nc


_NC_CACHE = None  # (key, nc)


def _make_in_maps(x, plan, cores, xgidx):
    x = np.ascontiguousarray(np.asarray(x, dtype=np.float32))
    in_maps = []
    for c in range(NCORES):
        bc, kc = divmod(c, 2)
        xsh = x[bc * BS : (bc + 1) * BS, :]
        xr = xsh[:, xgidx]  # (512 b, 128 nu, 60 t)
        xrep = np.ascontiguousarray(
            xr.transpose(1, 2, 0).reshape(128, 60 * BS).astype(np.float16)
        )
        co = cores[kc]
        A = (xsh[:, co["iks"]].T / 4.0).astype(np.float16)  # (NSLOT*128, 512)
        xg = np.ascontiguousarray(
            A.reshape(NSLOT, 128, BS).transpose(1, 0, 2).reshape(128, NSLOT * BS)
        )
        in_maps.append(
            {
                "xrep": xrep,
                "xg": xg,
                "sidx": co["sidx"],
                "wt0": co["wt0"],
                "wt1": co["wt1"],
                "wt23": co["wt23"],
            }
        )
    return in_maps


def kernel(x, W0, W1, W2, W3, bid0, bid1, bid2, bid3, W_invperm, **_unused):
    global _NC_CACHE
    plan, cores, xgidx = _host_prep(
        W0, W1, W2, W3, bid0, bid1, bid2, bid3, W_invperm
    )
    if _NC_CACHE is None or _NC_CACHE[0] != plan["key"]:
        _NC_CACHE = (plan["key"], _build_nc(plan))
    nc = _NC_CACHE[1]

    in_maps = _make_in_maps(x, plan, cores, xgidx)
    res = run_bass_kernel_spmd(nc, in_maps, core_ids=list(range(NCORES)))
    outs = [np.asarray(res.results[c]["out"], np.float32) for c in range(NCORES)]
    out = np.concatenate(
        [outs[2 * bc] + outs[2 * bc + 1] for bc in range(NCORES // 2)], axis=0
    )
    return out.astype(np.float32)


# revision 28
# speedup vs baseline: 1.0086x; 1.0086x over previous
"""Trainium2 Bass kernel for nn_EquivariantBiLinear.

Math (per batch row b):
    Y[k, b] = sum_nu W_g[mu, nu] * x[b, bid_g[nu*r+rho]]   (k = off_g + mu*r + rho)
    out[b, o] = 0.1 * sum_i Y[W_invperm[o*256+i], b] * x[b, i]

Sharding: 4-way over batch x 2-way over k-space (8 cores; host adds the
two k-partials per batch slice). Per core: 256 chunks of 128 k-rows x
512 batch cols. Per chunk: group GEMM (fp16, N=512) -> PSUM, DVE mult
by the host-gathered xg = x[b, i_k]/4 -> z (fp16), then a one-hot fp8
scatter matmul accumulates z into the persistent output PSUM bank(s).

Key trick vs the naive layout: the scatter previously needed 2 matmuls
per chunk (its 128 target o-rows span both 128-row PSUM banks). Since
the GEMM's mu-tiling within each (group, rho)-stream is free, the host
sorts each stream's k's by target o-half so almost every chunk is
*pure* (single-bank scatter: 1 matmul). Boundary/parity chunks stay
"mixed" (2 matmuls). All weights are streamed per-chunk in the sorted
order (pure layout transform on the host). The two k-cores must share
one SPMD program, so per (stream, class) chunk counts are made even by
demoting a few pure chunks to mixed; both cores then run an identical
slot sequence with different streamed data. Scatter matmuls drop from
512 to ~308 per core.

Scheduling: slots are ordered big(g0/g1)-small(g2/g3) alternating so
every PSUM-pair step carries enough PE work to hide the PSUM->SBUF
copy; xrep is loaded as 60 per-(group,rho) blocks posted in first-use
order so the first GEMM starts ~3us in; DMA fetches are batched into
>=1KB/partition slabs to cut sequencer post overhead. The o-half-0
output bank finishes before the pure-1 phase, so its epilogue overlaps
the remaining chunks.
"""

import sys

if "/opt/trn_rl_repo" not in sys.path:
    sys.path.insert(0, "/opt/trn_rl_repo")

from contextlib import ExitStack

import numpy as np

import concourse.bacc as bacc
import concourse.mybir as mybir
import concourse.tile as tile
from concourse.bass_utils import run_bass_kernel_spmd
from concourse.masks import make_identity

GROUPS = [(512, 1, 16384), (256, 4, 4096), (128, 16, 1024), (64, 64, 256)]
OFFS = [0, 16384, 32768, 49152]
X = 256
B = 2048
NCORES = 8
BS = 512  # batch rows per core
NSLOT = 256  # chunks per core

F32 = mybir.dt.float32
FP16 = mybir.dt.float16
FP8 = mybir.dt.float8e4

# earliest slot index per group (xrep warm-up)
GSTART = [4, 0, 20, 40]


def _streams(pairs3):
    """List of (g, stream_id, k-array in canonical order)."""
    out = []
    for gi, (n, r, m) in enumerate(GROUPS):
        off = OFFS[gi]
        if gi < 3:
            for rho in range(r):
                out.append((gi, rho, off + np.arange(m) * r + rho))
        else:
            for j, (sa, sb) in enumerate(pairs3):
                out.append(
                    (
                        gi,
                        j,
                        np.concatenate(
                            [
                                off + np.arange(m) * r + sa,
                                off + np.arange(m) * r + sb,
                            ]
                        ),
                    )
                )
    return out


def _pair_g3(perm):
    """Pair g3 rho-streams so each pair's o-half-0 count sums to exactly
    256 where possible: such pairs tile into 2+2 pure chunks with no
    mixed boundary and no parity demotion."""
    H = perm // (X * X // 2)
    off = OFFS[3]
    n0 = [
        int((H[off + np.arange(256) * 64 + s] == 0).sum()) for s in range(64)
    ]
    by_val = {}
    for s, v in enumerate(n0):
        by_val.setdefault(v, []).append(s)
    pairs = []
    used = [False] * 64
    for s in range(64):
        if used[s]:
            continue
        want = 256 - n0[s]
        cand = [t for t in by_val.get(want, []) if not used[t] and t != s]
        if cand:
            t = cand[0]
            used[s] = used[t] = True
            pairs.append((s, t))
    rest = [s for s in range(64) if not used[s]]
    rest.sort(key=lambda s: n0[s])
    while rest:
        pairs.append((rest.pop(0), rest.pop(-1)))
    assert len(pairs) == 32
    return pairs


def _order_slots(per_group):
    """Merge the 4 per-group item lists into one slot order: big (g0/g1)
    and small (g2/g3) alternate so PSUM pairs carry steady PE work;
    within each size class groups interleave proportionally; GSTART
    delays a group until its xrep blocks can be resident."""

    def mk_queue(groups):
        lists = {g: list(per_group[g]) for g in groups}
        tot = {g: max(1, len(lists[g])) for g in groups}
        idx = {g: 0 for g in groups}

        def pop(slot_i, force=False):
            best, bestv = None, -1.0
            for g in groups:
                if idx[g] >= len(lists[g]):
                    continue
                if slot_i < GSTART[g] and not force:
                    continue
                v = (len(lists[g]) - idx[g]) / tot[g]
                if v > bestv:
                    best, bestv = g, v
            if best is None:
                return None
            it = lists[best][idx[best]]
            idx[best] += 1
            return it

        return pop

    nb = len(per_group[0]) + len(per_group[1])
    ns = len(per_group[2]) + len(per_group[3])
    big = mk_queue([1, 0])
    small = mk_queue([2, 3])
    out = []
    want_big = False
    for _ in range(nb + ns):
        want_big = not want_big
        it = (big if want_big else small)(len(out))
        if it is None:
            it = (small if want_big else big)(len(out))
        if it is None:
            it = big(len(out), force=True) or small(len(out), force=True)
        out.append(it)
    return out


def _plan(perm):
    """Chunk the k-space into o-half-sorted 128-tiles and build the
    shared slot schedule."""
    H = perm // (X * X // 2)
    pairs3 = _pair_g3(perm)

    chunks = {}  # (g, s, cls) -> list of klist arrays
    for gi, s, ks in _streams(pairs3):
        hh = H[ks]
        order = np.argsort(hh, kind="stable")
        ks = ks[order]
        n0 = int((hh == 0).sum())
        a0, c = divmod(n0, 128)
        mx = 1 if c else 0
        a1 = (len(ks) - n0) // 128
        guard = 0
        while (a0 % 2) or (mx % 2) or (a1 % 2):
            if a0 % 2:
                a0 -= 1
                mx += 1
            elif a1 % 2:
                a1 -= 1
                mx += 1
            guard += 1
            assert guard < 8
        tiles = [ks[t * 128 : (t + 1) * 128] for t in range(len(ks) // 128)]
        chunks[(gi, s, 0)] = tiles[:a0]
        chunks[(gi, s, 1)] = tiles[a0 : a0 + mx]
        chunks[(gi, s, 2)] = tiles[a0 + mx :]

    chunk_of = [{}, {}]
    for key, lst in chunks.items():
        assert len(lst) % 2 == 0, (key, len(lst))
        chunk_of[0][key] = lst[0::2]
        chunk_of[1][key] = lst[1::2]

    per_group = {ph: {g: [] for g in range(4)} for ph in range(2)}
    for gi, s, _ks in _streams(pairs3):
        for cls in (0, 1, 2):
            nn = len(chunk_of[0][(gi, s, cls)])
            ph = 0 if cls < 2 else 1
            per_group[ph][gi].extend([(gi, s, cls)] * nn)

    # Blend bank0-touching (cls 0/1) and pure-1 slots into one stream so
    # the DMA-heavy mix is uniform (the old A-then-B split made the A
    # phase locally DMA-bound). Front-load bank0 work to finish by ~94%
    # of the timeline so its epilogue still overlaps remaining slots.
    merged = {}
    for g in range(4):
        A, Bl = per_group[0][g], per_group[1][g]
        a, b = len(A), len(Bl)
        n = a + b
        lst = []
        ai = bi = 0
        for i in range(n):
            take_a = ai < a and (
                bi >= b or ai / max(1, a) <= i / (0.94 * max(1, n))
            )
            if take_a:
                lst.append(A[ai])
                ai += 1
            else:
                lst.append(Bl[bi])
                bi += 1
        merged[g] = lst
    slots = _order_slots(merged)
    assert len(slots) == NSLOT

    meta = []
    i0 = i1 = i23 = 0
    icol = 0
    for si, (gi, s, cls) in enumerate(slots):
        if gi == 0:
            wt = ("wt0", i0 * 512)
            i0 += 1
        elif gi == 1:
            wt = ("wt1", i1 * 256)
            i1 += 1
        else:
            wt = ("wt23", i23 * 128)
            i23 += 1
        banks = [0] if cls == 0 else ([1] if cls == 2 else [0, 1])
        cols = {}
        for bk in banks:
            cols[bk] = icol
            icol += 1
        meta.append(dict(g=gi, s=s, cls=cls, wt=wt, banks=banks, icol=cols))
    nidx = (icol + 15) // 16 * 16

    first = {0: None, 1: None}
    last = {0: None, 1: None}
    for si, m in enumerate(meta):
        for bk in m["banks"]:
            if first[bk] is None:
                first[bk] = si
            last[bk] = si
    for si, m in enumerate(meta):
        m["start"] = {bk: si == first[bk] for bk in m["banks"]}
        m["stop"] = {bk: si == last[bk] for bk in m["banks"]}
    ep0_after = last[0]

    counts = (i0, i1, i23)
    return slots, chunk_of, meta, nidx, ep0_after, counts, pairs3


def _host_prep(W0, W1, W2, W3, bid0, bid1, bid2, bid3, W_invperm):
    """Pure layout transforms of weights/indices (no arithmetic on data)."""
    import ml_dtypes

    Ws = [np.asarray(W) for W in (W0, W1, W2, W3)]
    bids = [np.asarray(b).astype(np.int64) for b in (bid0, bid1, bid2, bid3)]
    ivp = np.asarray(W_invperm).astype(np.int64)
    perm = np.empty(X * X, np.int64)
    perm[ivp] = np.arange(X * X)

    slots, chunk_of, meta, nidx, ep0_after, counts, pairs3 = _plan(perm)

    wt = [
        np.ascontiguousarray(W.reshape(m, n).T.astype(np.float16))
        for (n, r, m), W in zip(GROUPS, Ws)
    ]

    pr = np.arange(128)
    cores = []
    for kc in range(2):
        qidx = {key: 0 for key in chunk_of[kc]}
        wt0p = np.zeros((128, counts[0] * 512), np.float16)
        wt1p = np.zeros((128, counts[1] * 256), np.float16)
        wt23p = np.zeros((128, counts[2] * 128), np.float16)
        sidx = np.full((128, nidx), -1.0, np.float32)
        iks = np.empty(NSLOT * 128, np.int64)
        for si, m in enumerate(meta):
            gi, s, cls = slots[si]
            ks = chunk_of[kc][(gi, s, cls)][qidx[(gi, s, cls)]]
            qidx[(gi, s, cls)] += 1
            off = OFFS[gi]
            n, r, _m = GROUPS[gi]
            kind, woff = m["wt"]
            if gi == 3:
                mu = (ks - off) // r
                rho = (ks - off) % r
                sub = np.where(rho == pairs3[s][0], 0, 1)
                assert np.all((rho == pairs3[s][0]) | (rho == pairs3[s][1]))
                blk = np.zeros((128, 128), np.float16)
                rows = sub[None, :] * 64 + np.arange(64)[:, None]  # (64, 128)
                blk[rows, np.arange(128)[None, :]] = wt[3][:, mu]
                wt23p[:, woff : woff + 128] = blk
            else:
                mu = (ks - off) // r
                cols = wt[gi][:, mu]  # (n, 128)
                if gi == 0:
                    for kcc in range(4):
                        wt0p[:, woff + kcc * 128 : woff + (kcc + 1) * 128] = cols[
                            kcc * 128 : (kcc + 1) * 128, :
                        ]
                elif gi == 1:
                    for kcc in range(2):
                        wt1p[:, woff + kcc * 128 : woff + (kcc + 1) * 128] = cols[
                            kcc * 128 : (kcc + 1) * 128, :
                        ]
                else:
                    wt23p[:, woff : woff + 128] = cols
            pk = perm[ks]
            iks[si * 128 : (si + 1) * 128] = pk % X
            ok = pk // X
            for bk, col in m["icol"].items():
                sel = (ok // 128) == bk
                sidx[pr[sel], col] = (ok % 128)[sel]
        cores.append(dict(wt0=wt0p, wt1=wt1p, wt23=wt23p, sidx=sidx, iks=iks))

    # x-gather column map for xrep (identical to reference layout)
    colsl = []
    b0 = bids[0]
    for kcc in range(4):
        colsl.append(b0[kcc * 128 : (kcc + 1) * 128])
    b1 = bids[1].reshape(256, 4)
    for rho in range(4):
        for kcc in range(2):
            colsl.append(b1[kcc * 128 : (kcc + 1) * 128, rho])
    b2 = bids[2].reshape(128, 16)
    for rho in range(16):
        colsl.append(b2[:, rho])
    b3 = bids[3].reshape(64, 64)
    for sa, sb in pairs3:
        colsl.append(b3[pr % 64, np.where(pr // 64 == 0, sa, sb)])
    xgidx = np.ascontiguousarray(np.stack(colsl, axis=1).astype(np.int64))

    plan_key = (
        tuple(tuple(p) for p in pairs3),
        tuple(slots),
        tuple(tuple(sorted(m["icol"].items())) for m in meta),
        nidx,
        ep0_after,
        counts,
    )
    plan = dict(
        slots=slots, meta=meta, nidx=nidx, ep0_after=ep0_after, counts=counts,
        key=hash(repr(plan_key)),
    )
    return plan, cores, xgidx


def _xrep_block(gi, s, kcc=0):
    """xrep plane block index for (group, stream[, kc])."""
    if gi == 0:
        return kcc
    if gi == 1:
        return 4 + s * 2 + kcc
    if gi == 2:
        return 12 + s
    return 28 + s


def _build_nc(plan):
    meta = plan["meta"]
    nidx = plan["nidx"]
    ep0_after = plan["ep0_after"]
    n0s, n1s, n23s = plan["counts"]

    nc = bacc.Bacc("TRN2", target_bir_lowering=False, debug=False, num_devices=NCORES)

    xrep_d = nc.dram_tensor("xrep", [128, 60 * BS], FP16, kind="ExternalInput")
    xg_d = nc.dram_tensor("xg", [128, NSLOT * BS], FP16, kind="ExternalInput")
    wt0_d = nc.dram_tensor("wt0", [128, n0s * 512], FP16, kind="ExternalInput")
    wt1_d = nc.dram_tensor("wt1", [128, n1s * 256], FP16, kind="ExternalInput")
    wt23_d = nc.dram_tensor("wt23", [128, n23s * 128], FP16, kind="ExternalInput")
    sidx_d = nc.dram_tensor("sidx", [128, nidx], F32, kind="ExternalInput")
    out_d = nc.dram_tensor("out", [BS, X], F32, kind="ExternalOutput")

    # ---- fetch schedule: batched slabs, attached to the pair that is
    # PF pairs ahead of first use (negative -> preamble) ----
    PF = 6
    npair = NSLOT // 2
    fetch = [[] for _ in range(npair)]
    pre = []  # preamble fetches, in priority order

    def sched(first_use_pair, op, rank=3):
        p = first_use_pair - PF
        if p < 0:
            pre.append(((first_use_pair, rank), op))
        else:
            fetch[p].append(op)

    # xrep fetch units: g0 one [128,4BS]; g1 per-stream [128,2BS];
    # g2 stream-pairs [128,2BS]; g3 stream-quads [128,4BS]
    xrep_first = {}
    for si, m in enumerate(meta):
        gi, s = m["g"], m["s"]
        if gi == 0:
            unit = ("xr0", 0)
        elif gi == 1:
            unit = ("xr1", s)
        elif gi == 2:
            unit = ("xr2", s // 2)
        else:
            unit = ("xr3", s // 4)
        if unit not in xrep_first:
            xrep_first[unit] = (si // 2, si)
    for unit, (fu, _fs) in sorted(xrep_first.items(), key=lambda kv: kv[1]):
        sched(fu, unit, rank=1)

    # xg slabs of 4 pairs
    for slab in range((npair + 3) // 4):
        sched(slab * 4, ("xg", slab), rank=4)
    # wt windows: wt0 [4 g0-slots], wt1 [4 g1-slots], wt23 [16 slots]
    seen = set()
    for si, m in enumerate(meta):
        kind, woff = m["wt"]
        w = woff // 2048 if kind != "wt1" else woff // 1024
        if (kind, w) not in seen:
            seen.add((kind, w))
            sched(si // 2, (kind, w), rank=0 if kind == "wt1" else 2)
    sched(0, ("sidx", 0), rank=5)
    pre.sort(key=lambda kv: kv[0])

    with tile.TileContext(nc) as tc, ExitStack() as ctx:
        const = ctx.enter_context(tc.tile_pool(name="const", bufs=1))
        w0pool = ctx.enter_context(tc.tile_pool(name="w0pool", bufs=4))
        w1pool = ctx.enter_context(tc.tile_pool(name="w1pool", bufs=4))
        w23pool = ctx.enter_context(tc.tile_pool(name="w23pool", bufs=4))
        xgpool = ctx.enter_context(tc.tile_pool(name="xgpool", bufs=4))
        ohpool = ctx.enter_context(tc.tile_pool(name="ohpool", bufs=16))
        ypool = ctx.enter_context(tc.tile_pool(name="ypool", bufs=4))
        zpool = ctx.enter_context(tc.tile_pool(name="zpool", bufs=8))
        pgemm = ctx.enter_context(tc.tile_pool(name="pgemm", bufs=3, space="PSUM"))
        pout = ctx.enter_context(tc.tile_pool(name="pout", bufs=1, space="PSUM"))

        ident = const.tile([128, 128], F32)
        make_identity(nc, ident[:])
        # PE p-state warm-up: the tensor engine needs ~3us of continuous
        # execution to reach full clock. Run throwaway matmuls while the
        # first real fetches are still in flight so the ramp happens in
        # otherwise-idle time and slot 0 starts at full speed.
        warm = const.tile([128, 512], FP16, name="warm")
        nc.gpsimd.memset(warm[:], 0.0)
        wps = pgemm.tile([128, 1024], F32, tag="pg", name="warmps")
        for _wi in range(6):
            nc.tensor.matmul(
                wps[:, 0:512],
                warm[:, 0:128],
                warm[:],
                start=True,
                stop=True,
                skip_group_check=True,
            )
        iota_t = const.tile([128, 128], FP16, name="iota")
        nc.gpsimd.iota(
            iota_t[:],
            pattern=[[1, 128]],
            base=0,
            channel_multiplier=0,
            allow_small_or_imprecise_dtypes=True,
        )
        sidxt = const.tile([128, nidx], F32, name="sidxt")

        xr0t = const.tile([128, 4 * BS], FP16, name="xr0t")
        xr1t = [const.tile([128, 2 * BS], FP16, name=f"xr1_{s}") for s in range(4)]
        xr2t = [const.tile([128, 2 * BS], FP16, name=f"xr2_{j}") for j in range(8)]
        xr3t = [const.tile([128, 4 * BS], FP16, name=f"xr3_{j}") for j in range(8)]

        # All steady-state posts go to the sync sequencer: descriptors of
        # any one post spread across all 16 DMA engines, so a single ring
        # sustains full bandwidth; posting from Act would head-of-line
        # block the PSUM->SBUF copies behind pool-buffer semaphore waits.
        rb = {"n": 0, "startup": True}

        def ring(nbytes):
            if rb["startup"]:
                rb["n"] += 1
                return nc.scalar if rb["n"] % 2 else nc.sync
            return nc.sync

        state = {
            "xgq": [], "w0q": [], "w1q": [], "w23q": [],
            "xg_cur": None, "xg_off": 0,
            "w0_cur": None, "w0_off": 4, "w1_cur": None, "w1_off": 4,
            "w23_cur": None, "w23_off": 16,
            "pend": [], "ps": None,
        }

        def do_op(op):
            kind, arg = op
            if kind == "xr0":
                ring(4 * BS * 256).dma_start(xr0t[:], xrep_d[:, 0 : 4 * BS])
            elif kind == "xr1":
                lo = (4 + arg * 2) * BS
                ring(2 * BS * 256).dma_start(
                    xr1t[arg][:], xrep_d[:, lo : lo + 2 * BS]
                )
            elif kind == "xr2":
                lo = (12 + arg * 2) * BS
                ring(2 * BS * 256).dma_start(
                    xr2t[arg][:], xrep_d[:, lo : lo + 2 * BS]
                )
            elif kind == "xr3":
                lo = (28 + arg * 4) * BS
                ring(4 * BS * 256).dma_start(
                    xr3t[arg][:], xrep_d[:, lo : lo + 4 * BS]
                )
            elif kind == "sidx":
                ring(nidx * 512).dma_start(sidxt[:], sidx_d[:])
            elif kind == "xg":
                t = xgpool.tile([128, 4096], FP16, tag="xgt", name="xgt")
                lo = arg * 4096
                hi = min(lo + 4096, NSLOT * BS)
                ring((hi - lo) * 256).dma_start(t[:, : hi - lo], xg_d[:, lo:hi])
                state["xgq"].append(t)
            elif kind == "wt0":
                t = w0pool.tile([128, 2048], FP16, tag="w0t", name="w0t")
                lo = arg * 2048
                hi = min(lo + 2048, n0s * 512)
                ring((hi - lo) * 256).dma_start(t[:, : hi - lo], wt0_d[:, lo:hi])
                state["w0q"].append(t)
            elif kind == "wt1":
                t = w1pool.tile([128, 1024], FP16, tag="w1t", name="w1t")
                lo = arg * 1024
                hi = min(lo + 1024, n1s * 256)
                ring((hi - lo) * 256).dma_start(t[:, : hi - lo], wt1_d[:, lo:hi])
                state["w1q"].append(t)
            elif kind == "wt23":
                t = w23pool.tile([128, 2048], FP16, tag="w23t", name="w23t")
                lo = arg * 2048
                hi = min(lo + 2048, n23s * 128)
                ring((hi - lo) * 256).dma_start(t[:, : hi - lo], wt23_d[:, lo:hi])
                state["w23q"].append(t)

        outT_ps = [
            pout.tile([128, BS], F32, tag=f"pout{h}", name=f"pout{h}") for h in range(2)
        ]

        def flush_pending():
            for si, ohts, zv in state["pend"]:
                m = meta[si]
                for bk, lhs in ohts:
                    nc.tensor.matmul(
                        outT_ps[bk][:],
                        lhs,
                        zv,
                        start=m["start"][bk],
                        stop=m["stop"][bk],
                        skip_group_check=True,
                    )
            state["pend"].clear()

        def epilogue(hb):
            # scale out of the bank, transpose back INTO the drained bank
            # (no pgemm-pool churn), stage once to SBUF, then DMA out.
            outT_sb = zpool.tile([128, BS], F32, tag="outT_sb", name="outT_sb", bufs=2)
            nc.scalar.mul(outT_sb[:], outT_ps[hb][:], 0.4)
            for bh in range(4):
                nc.tensor.transpose(
                    outT_ps[hb][:, bh * 128 : (bh + 1) * 128],
                    outT_sb[:, bh * 128 : (bh + 1) * 128],
                    ident[:],
                )
            stg = zpool.tile([128, BS], F32, tag="outstg", name="outstg", bufs=2)
            nc.scalar.copy(stg[:], outT_ps[hb][:])
            for bh in range(4):
                nc.sync.dma_start(
                    out_d[bh * 128 : (bh + 1) * 128, hb * 128 : (hb + 1) * 128],
                    stg[:, bh * 128 : (bh + 1) * 128],
                )

        # ---- preamble fetches (startup-critical, ring-alternating) ----
        for _key, op in pre:
            do_op(op)
        rb["startup"] = False

        # ---- main loop over slots ----
        for si, m in enumerate(meta):
            gi = m["g"]
            s = m["s"]
            if si % 2 == 0:
                for op in fetch[si // 2]:
                    do_op(op)
                ps = pgemm.tile([128, 1024], F32, tag="pg", name="ps")
                state["ps"] = ps
            ps_half = state["ps"][:, (si % 2) * 512 : (si % 2 + 1) * 512]

            # GEMM
            if gi == 0:
                if state["w0_off"] == 4:
                    state["w0_cur"] = state["w0q"].pop(0)
                    state["w0_off"] = 0
                w = state["w0_cur"]
                o = state["w0_off"] * 512
                state["w0_off"] += 1
                for kcc in range(4):
                    nc.tensor.matmul(
                        ps_half,
                        w[:, o + kcc * 128 : o + (kcc + 1) * 128],
                        xr0t[:, kcc * BS : (kcc + 1) * BS],
                        start=(kcc == 0),
                        stop=(kcc == 3),
                    )
            elif gi == 1:
                if state["w1_off"] == 4:
                    state["w1_cur"] = state["w1q"].pop(0)
                    state["w1_off"] = 0
                w = state["w1_cur"]
                o = state["w1_off"] * 256
                state["w1_off"] += 1
                for kcc in range(2):
                    nc.tensor.matmul(
                        ps_half,
                        w[:, o + kcc * 128 : o + (kcc + 1) * 128],
                        xr1t[s][:, kcc * BS : (kcc + 1) * BS],
                        start=(kcc == 0),
                        stop=(kcc == 1),
                    )
            else:
                if state["w23_off"] == 16:
                    state["w23_cur"] = state["w23q"].pop(0)
                    state["w23_off"] = 0
                w = state["w23_cur"]
                o = state["w23_off"] * 128
                state["w23_off"] += 1
                rhs = (
                    xr2t[s // 2][:, (s % 2) * BS : (s % 2 + 1) * BS]
                    if gi == 2
                    else xr3t[s // 4][:, (s % 4) * BS : (s % 4 + 1) * BS]
                )
                nc.tensor.matmul(
                    ps_half,
                    w[:, o : o + 128],
                    rhs,
                    start=True,
                    stop=True,
                )

            if si % 2 == 1:
                thresh = 2 if si >= NSLOT - 16 else 6
                if len(state["pend"]) >= thresh:
                    flush_pending()
                pair = si // 2
                if state["xg_off"] == 4 or state["xg_cur"] is None:
                    state["xg_cur"] = state["xgq"].pop(0)
                    state["xg_off"] = 0
                xgt = state["xg_cur"]
                xo = state["xg_off"] * 1024
                state["xg_off"] += 1
                z16 = zpool.tile([128, 1024], FP16, tag="z16", name="z16")
                if pair >= NSLOT // 2 - 5:
                    # tail pairs: fused PSUM-read multiply shortens the
                    # end-of-stream copy->mult->scatter chain
                    nc.vector.tensor_mul(
                        z16[:], state["ps"][:], xgt[:, xo : xo + 1024]
                    )
                else:
                    yt = ypool.tile([128, 1024], FP16, tag="yt", name="yt")
                    if pair < 6:
                        nc.vector.tensor_copy(yt[:], state["ps"][:])
                    else:
                        nc.scalar.copy(yt[:], state["ps"][:])
                    nc.vector.tensor_mul(z16[:], yt[:], xgt[:, xo : xo + 1024])
                for j in (si - 1, si):
                    mm = meta[j]
                    ohts = []
                    for bk in mm["banks"]:
                        col = mm["icol"][bk]
                        ohg = ohpool.tile([128, 128], FP16, tag="ohg", name="ohg")
                        nc.vector.tensor_scalar(
                            ohg[:],
                            iota_t[:],
                            sidxt[:, col : col + 1],
                            None,
                            mybir.AluOpType.is_equal,
                        )
                        ohts.append((bk, ohg[:]))
                    zv = z16[:, (j % 2) * 512 : (j % 2 + 1) * 512]
                    state["pend"].append((j, ohts, zv))
                if si == ep0_after or si - 1 == ep0_after:
                    flush_pending()
                    epilogue(0)

        flush_pending()
        epilogue(1)

    nc.compile()
    return nc


_NC_CACHE = None  # (key, nc)


def _make_in_maps(x, plan, cores, xgidx):
    x = np.ascontiguousarray(np.asarray(x, dtype=np.float32))
    in_maps = []
    for c in range(NCORES):
        bc, kc = divmod(c, 2)
        xsh = x[bc * BS : (bc + 1) * BS, :]
        xr = xsh[:, xgidx]  # (512 b, 128 nu, 60 t)
        xrep = np.ascontiguousarray(
            xr.transpose(1, 2, 0).reshape(128, 60 * BS).astype(np.float16)
        )
        co = cores[kc]
        A = (xsh[:, co["iks"]].T / 4.0).astype(np.float16)  # (NSLOT*128, 512)
        xg = np.ascontiguousarray(
            A.reshape(NSLOT, 128, BS).transpose(1, 0, 2).reshape(128, NSLOT * BS)
        )
        in_maps.append(
            {
                "xrep": xrep,
                "xg": xg,
                "sidx": co["sidx"],
                "wt0": co["wt0"],
                "wt1": co["wt1"],
                "wt23": co["wt23"],
            }
        )
    return in_maps


def kernel(x, W0, W1, W2, W3, bid0, bid1, bid2, bid3, W_invperm, **_unused):
    global _NC_CACHE
    plan, cores, xgidx = _host_prep(
        W0, W1, W2, W3, bid0, bid1, bid2, bid3, W_invperm
    )
    if _NC_CACHE is None or _NC_CACHE[0] != plan["key"]:
        _NC_CACHE = (plan["key"], _build_nc(plan))
    nc = _NC_CACHE[1]

    in_maps = _make_in_maps(x, plan, cores, xgidx)
    res = run_bass_kernel_spmd(nc, in_maps, core_ids=list(range(NCORES)))
    outs = [np.asarray(res.results[c]["out"], np.float32) for c in range(NCORES)]
    out = np.concatenate(
        [outs[2 * bc] + outs[2 * bc + 1] for bc in range(NCORES // 2)], axis=0
    )
    return out.astype(np.float32)


# revision 32
# speedup vs baseline: 1.0176x; 1.0089x over previous
"""Trainium2 Bass kernel for nn_EquivariantBiLinear.

Math (per batch row b):
    Y[k, b] = sum_nu W_g[mu, nu] * x[b, bid_g[nu*r+rho]]   (k = off_g + mu*r + rho)
    out[b, o] = 0.1 * sum_i Y[W_invperm[o*256+i], b] * x[b, i]

Sharding: 4-way over batch x 2-way over k-space (8 cores; host adds the
two k-partials per batch slice). Per core: 256 chunks of 128 k-rows x
512 batch cols. Per chunk: group GEMM (fp16, N=512) -> PSUM, DVE mult
by the host-gathered xg = x[b, i_k]/4 -> z (fp16), then a one-hot fp8
scatter matmul accumulates z into the persistent output PSUM bank(s).

Key trick vs the naive layout: the scatter previously needed 2 matmuls
per chunk (its 128 target o-rows span both 128-row PSUM banks). Since
the GEMM's mu-tiling within each (group, rho)-stream is free, the host
sorts each stream's k's by target o-half so almost every chunk is
*pure* (single-bank scatter: 1 matmul). Boundary/parity chunks stay
"mixed" (2 matmuls). All weights are streamed per-chunk in the sorted
order (pure layout transform on the host). The two k-cores must share
one SPMD program, so per (stream, class) chunk counts are made even by
demoting a few pure chunks to mixed; both cores then run an identical
slot sequence with different streamed data. Scatter matmuls drop from
512 to ~308 per core.

Scheduling: slots are ordered big(g0/g1)-small(g2/g3) alternating so
every PSUM-pair step carries enough PE work to hide the PSUM->SBUF
copy; xrep is loaded as 60 per-(group,rho) blocks posted in first-use
order so the first GEMM starts ~3us in; DMA fetches are batched into
>=1KB/partition slabs to cut sequencer post overhead. The o-half-0
output bank finishes before the pure-1 phase, so its epilogue overlaps
the remaining chunks.
"""

import sys

if "/opt/trn_rl_repo" not in sys.path:
    sys.path.insert(0, "/opt/trn_rl_repo")

from contextlib import ExitStack

import numpy as np

import concourse.bacc as bacc
import concourse.mybir as mybir
import concourse.tile as tile
from concourse.bass_utils import run_bass_kernel_spmd
from concourse.masks import make_identity

GROUPS = [(512, 1, 16384), (256, 4, 4096), (128, 16, 1024), (64, 64, 256)]
OFFS = [0, 16384, 32768, 49152]
X = 256
B = 2048
NCORES = 8
BS = 512  # batch rows per core
NSLOT = 256  # chunks per core

F32 = mybir.dt.float32
FP16 = mybir.dt.float16
FP8 = mybir.dt.float8e4

# earliest slot index per group (xrep warm-up)
GSTART = [4, 0, 20, 40]


def _streams(pairs3):
    """List of (g, stream_id, k-array in canonical order)."""
    out = []
    for gi, (n, r, m) in enumerate(GROUPS):
        off = OFFS[gi]
        if gi < 3:
            for rho in range(r):
                out.append((gi, rho, off + np.arange(m) * r + rho))
        else:
            for j, (sa, sb) in enumerate(pairs3):
                out.append(
                    (
                        gi,
                        j,
                        np.concatenate(
                            [
                                off + np.arange(m) * r + sa,
                                off + np.arange(m) * r + sb,
                            ]
                        ),
                    )
                )
    return out


def _pair_g3(perm):
    """Pair g3 rho-streams so each pair's o-half-0 count sums to exactly
    256 where possible: such pairs tile into 2+2 pure chunks with no
    mixed boundary and no parity demotion."""
    H = perm // (X * X // 2)
    off = OFFS[3]
    n0 = [
        int((H[off + np.arange(256) * 64 + s] == 0).sum()) for s in range(64)
    ]
    by_val = {}
    for s, v in enumerate(n0):
        by_val.setdefault(v, []).append(s)
    pairs = []
    used = [False] * 64
    for s in range(64):
        if used[s]:
            continue
        want = 256 - n0[s]
        cand = [t for t in by_val.get(want, []) if not used[t] and t != s]
        if cand:
            t = cand[0]
            used[s] = used[t] = True
            pairs.append((s, t))
    rest = [s for s in range(64) if not used[s]]
    rest.sort(key=lambda s: n0[s])
    while rest:
        pairs.append((rest.pop(0), rest.pop(-1)))
    assert len(pairs) == 32
    return pairs


def _order_slots(per_group):
    """Merge the 4 per-group item lists into one slot order: big (g0/g1)
    and small (g2/g3) alternate so PSUM pairs carry steady PE work;
    within each size class groups interleave proportionally; GSTART
    delays a group until its xrep blocks can be resident."""

    def mk_queue(groups):
        lists = {g: list(per_group[g]) for g in groups}
        tot = {g: max(1, len(lists[g])) for g in groups}
        idx = {g: 0 for g in groups}

        def pop(slot_i, force=False):
            best, bestv = None, -1.0
            for g in groups:
                if idx[g] >= len(lists[g]):
                    continue
                if slot_i < GSTART[g] and not force:
                    continue
                v = (len(lists[g]) - idx[g]) / tot[g]
                if v > bestv:
                    best, bestv = g, v
            if best is None:
                return None
            it = lists[best][idx[best]]
            idx[best] += 1
            return it

        return pop

    nb = len(per_group[0]) + len(per_group[1])
    ns = len(per_group[2]) + len(per_group[3])
    big = mk_queue([1, 0])
    small = mk_queue([2, 3])
    out = []
    want_big = False
    for _ in range(nb + ns):
        want_big = not want_big
        it = (big if want_big else small)(len(out))
        if it is None:
            it = (small if want_big else big)(len(out))
        if it is None:
            it = big(len(out), force=True) or small(len(out), force=True)
        out.append(it)
    return out


def _plan(perm):
    """Chunk the k-space into o-half-sorted 128-tiles and build the
    shared slot schedule."""
    H = perm // (X * X // 2)
    pairs3 = _pair_g3(perm)

    chunks = {}  # (g, s, cls) -> list of klist arrays
    for gi, s, ks in _streams(pairs3):
        hh = H[ks]
        order = np.argsort(hh, kind="stable")
        ks = ks[order]
        n0 = int((hh == 0).sum())
        a0, c = divmod(n0, 128)
        mx = 1 if c else 0
        a1 = (len(ks) - n0) // 128
        guard = 0
        while (a0 % 2) or (mx % 2) or (a1 % 2):
            if a0 % 2:
                a0 -= 1
                mx += 1
            elif a1 % 2:
                a1 -= 1
                mx += 1
            guard += 1
            assert guard < 8
        tiles = [ks[t * 128 : (t + 1) * 128] for t in range(len(ks) // 128)]
        chunks[(gi, s, 0)] = tiles[:a0]
        chunks[(gi, s, 1)] = tiles[a0 : a0 + mx]
        chunks[(gi, s, 2)] = tiles[a0 + mx :]

    chunk_of = [{}, {}]
    for key, lst in chunks.items():
        assert len(lst) % 2 == 0, (key, len(lst))
        chunk_of[0][key] = lst[0::2]
        chunk_of[1][key] = lst[1::2]

    per_group = {ph: {g: [] for g in range(4)} for ph in range(2)}
    for gi, s, _ks in _streams(pairs3):
        for cls in (0, 1, 2):
            nn = len(chunk_of[0][(gi, s, cls)])
            ph = 0 if cls < 2 else 1
            per_group[ph][gi].extend([(gi, s, cls)] * nn)

    # Blend bank0-touching (cls 0/1) and pure-1 slots into one stream so
    # the DMA-heavy mix is uniform (the old A-then-B split made the A
    # phase locally DMA-bound). Front-load bank0 work to finish by ~94%
    # of the timeline so its epilogue still overlaps remaining slots.
    merged = {}
    for g in range(4):
        A, Bl = per_group[0][g], per_group[1][g]
        a, b = len(A), len(Bl)
        n = a + b
        lst = []
        ai = bi = 0
        for i in range(n):
            take_a = ai < a and (
                bi >= b or ai / max(1, a) <= i / (0.94 * max(1, n))
            )
            if take_a:
                lst.append(A[ai])
                ai += 1
            else:
                lst.append(Bl[bi])
                bi += 1
        merged[g] = lst
    slots = _order_slots(merged)
    assert len(slots) == NSLOT

    meta = []
    i0 = i1 = i23 = 0
    icol = 0
    for si, (gi, s, cls) in enumerate(slots):
        if gi == 0:
            wt = ("wt0", i0 * 512)
            i0 += 1
        elif gi == 1:
            wt = ("wt1", i1 * 256)
            i1 += 1
        else:
            wt = ("wt23", i23 * 128)
            i23 += 1
        banks = [0] if cls == 0 else ([1] if cls == 2 else [0, 1])
        cols = {}
        for bk in banks:
            cols[bk] = icol
            icol += 1
        meta.append(dict(g=gi, s=s, cls=cls, wt=wt, banks=banks, icol=cols))
    nidx = (icol + 15) // 16 * 16

    first = {0: None, 1: None}
    last = {0: None, 1: None}
    for si, m in enumerate(meta):
        for bk in m["banks"]:
            if first[bk] is None:
                first[bk] = si
            last[bk] = si
    for si, m in enumerate(meta):
        m["start"] = {bk: si == first[bk] for bk in m["banks"]}
        m["stop"] = {bk: si == last[bk] for bk in m["banks"]}
    ep0_after = last[0]

    counts = (i0, i1, i23)
    return slots, chunk_of, meta, nidx, ep0_after, counts, pairs3


def _host_prep(W0, W1, W2, W3, bid0, bid1, bid2, bid3, W_invperm):
    """Pure layout transforms of weights/indices (no arithmetic on data)."""
    import ml_dtypes

    Ws = [np.asarray(W) for W in (W0, W1, W2, W3)]
    bids = [np.asarray(b).astype(np.int64) for b in (bid0, bid1, bid2, bid3)]
    ivp = np.asarray(W_invperm).astype(np.int64)
    perm = np.empty(X * X, np.int64)
    perm[ivp] = np.arange(X * X)

    slots, chunk_of, meta, nidx, ep0_after, counts, pairs3 = _plan(perm)

    wt = [
        np.ascontiguousarray(W.reshape(m, n).T.astype(np.float16))
        for (n, r, m), W in zip(GROUPS, Ws)
    ]

    pr = np.arange(128)
    cores = []
    for kc in range(2):
        qidx = {key: 0 for key in chunk_of[kc]}
        wt0p = np.zeros((128, counts[0] * 512), np.float16)
        wt1p = np.zeros((128, counts[1] * 256), np.float16)
        wt23p = np.zeros((128, counts[2] * 128), np.float16)
        sidx = np.full((128, nidx), -1.0, np.float32)
        iks = np.empty(NSLOT * 128, np.int64)
        for si, m in enumerate(meta):
            gi, s, cls = slots[si]
            ks = chunk_of[kc][(gi, s, cls)][qidx[(gi, s, cls)]]
            qidx[(gi, s, cls)] += 1
            off = OFFS[gi]
            n, r, _m = GROUPS[gi]
            kind, woff = m["wt"]
            if gi == 3:
                mu = (ks - off) // r
                rho = (ks - off) % r
                sub = np.where(rho == pairs3[s][0], 0, 1)
                assert np.all((rho == pairs3[s][0]) | (rho == pairs3[s][1]))
                blk = np.zeros((128, 128), np.float16)
                rows = sub[None, :] * 64 + np.arange(64)[:, None]  # (64, 128)
                blk[rows, np.arange(128)[None, :]] = wt[3][:, mu]
                wt23p[:, woff : woff + 128] = blk
            else:
                mu = (ks - off) // r
                cols = wt[gi][:, mu]  # (n, 128)
                if gi == 0:
                    for kcc in range(4):
                        wt0p[:, woff + kcc * 128 : woff + (kcc + 1) * 128] = cols[
                            kcc * 128 : (kcc + 1) * 128, :
                        ]
                elif gi == 1:
                    for kcc in range(2):
                        wt1p[:, woff + kcc * 128 : woff + (kcc + 1) * 128] = cols[
                            kcc * 128 : (kcc + 1) * 128, :
                        ]
                else:
                    wt23p[:, woff : woff + 128] = cols
            pk = perm[ks]
            iks[si * 128 : (si + 1) * 128] = pk % X
            ok = pk // X
            for bk, col in m["icol"].items():
                sel = (ok // 128) == bk
                sidx[pr[sel], col] = (ok % 128)[sel]
        cores.append(dict(wt0=wt0p, wt1=wt1p, wt23=wt23p, sidx=sidx, iks=iks))

    # x-gather column map for xrep (identical to reference layout)
    colsl = []
    b0 = bids[0]
    for kcc in range(4):
        colsl.append(b0[kcc * 128 : (kcc + 1) * 128])
    b1 = bids[1].reshape(256, 4)
    for rho in range(4):
        for kcc in range(2):
            colsl.append(b1[kcc * 128 : (kcc + 1) * 128, rho])
    b2 = bids[2].reshape(128, 16)
    for rho in range(16):
        colsl.append(b2[:, rho])
    b3 = bids[3].reshape(64, 64)
    for sa, sb in pairs3:
        colsl.append(b3[pr % 64, np.where(pr // 64 == 0, sa, sb)])
    xgidx = np.ascontiguousarray(np.stack(colsl, axis=1).astype(np.int64))

    plan_key = (
        tuple(tuple(p) for p in pairs3),
        tuple(slots),
        tuple(tuple(sorted(m["icol"].items())) for m in meta),
        nidx,
        ep0_after,
        counts,
    )
    plan = dict(
        slots=slots, meta=meta, nidx=nidx, ep0_after=ep0_after, counts=counts,
        key=hash(repr(plan_key)),
    )
    return plan, cores, xgidx


def _xrep_block(gi, s, kcc=0):
    """xrep plane block index for (group, stream[, kc])."""
    if gi == 0:
        return kcc
    if gi == 1:
        return 4 + s * 2 + kcc
    if gi == 2:
        return 12 + s
    return 28 + s


def _build_nc(plan):
    meta = plan["meta"]
    nidx = plan["nidx"]
    ep0_after = plan["ep0_after"]
    n0s, n1s, n23s = plan["counts"]

    nc = bacc.Bacc("TRN2", target_bir_lowering=False, debug=False, num_devices=NCORES)

    xrep_d = nc.dram_tensor("xrep", [128, 60 * BS], FP16, kind="ExternalInput")
    xg_d = nc.dram_tensor("xg", [128, NSLOT * BS], FP16, kind="ExternalInput")
    wt0_d = nc.dram_tensor("wt0", [128, n0s * 512], FP16, kind="ExternalInput")
    wt1_d = nc.dram_tensor("wt1", [128, n1s * 256], FP16, kind="ExternalInput")
    wt23_d = nc.dram_tensor("wt23", [128, n23s * 128], FP16, kind="ExternalInput")
    sidx_d = nc.dram_tensor("sidx", [128, nidx], F32, kind="ExternalInput")
    out_d = nc.dram_tensor("out", [BS, X], F32, kind="ExternalOutput")

    # ---- fetch schedule: batched slabs, attached to the pair that is
    # PF pairs ahead of first use (negative -> preamble) ----
    PF = 6
    npair = NSLOT // 2
    fetch = [[] for _ in range(npair)]
    pre = []  # preamble fetches, in priority order

    def sched(first_use_pair, op, rank=3):
        p = first_use_pair - PF
        if p < 0:
            pre.append(((first_use_pair, rank), op))
        else:
            fetch[p].append(op)

    # xrep fetch units: g0 one [128,4BS]; g1 per-stream [128,2BS];
    # g2 stream-pairs [128,2BS]; g3 stream-quads [128,4BS]
    xrep_first = {}
    for si, m in enumerate(meta):
        gi, s = m["g"], m["s"]
        if gi == 0:
            unit = ("xr0", 0)
        elif gi == 1:
            unit = ("xr1", s)
        elif gi == 2:
            unit = ("xr2", s // 2)
        else:
            unit = ("xr3", s // 4)
        if unit not in xrep_first:
            xrep_first[unit] = (si // 2, si)
    for unit, (fu, _fs) in sorted(xrep_first.items(), key=lambda kv: kv[1]):
        sched(fu, unit, rank=1)

    # xg slabs of 4 pairs
    for slab in range((npair + 3) // 4):
        sched(slab * 4, ("xg", slab), rank=4)
    # wt windows: wt0 [4 g0-slots], wt1 [4 g1-slots], wt23 [16 slots]
    seen = set()
    for si, m in enumerate(meta):
        kind, woff = m["wt"]
        w = woff // 2048 if kind != "wt1" else woff // 1024
        if (kind, w) not in seen:
            seen.add((kind, w))
            sched(si // 2, (kind, w), rank=0 if kind == "wt1" else 2)
    sched(0, ("sidx", 0), rank=5)
    pre.sort(key=lambda kv: kv[0])

    with tile.TileContext(nc) as tc, ExitStack() as ctx:
        const = ctx.enter_context(tc.tile_pool(name="const", bufs=1))
        w0pool = ctx.enter_context(tc.tile_pool(name="w0pool", bufs=5))
        w1pool = ctx.enter_context(tc.tile_pool(name="w1pool", bufs=6))
        w23pool = ctx.enter_context(tc.tile_pool(name="w23pool", bufs=5))
        xgpool = ctx.enter_context(tc.tile_pool(name="xgpool", bufs=5))
        ohpool = ctx.enter_context(tc.tile_pool(name="ohpool", bufs=16))
        ypool = ctx.enter_context(tc.tile_pool(name="ypool", bufs=4))
        zpool = ctx.enter_context(tc.tile_pool(name="zpool", bufs=8))
        pgemm = ctx.enter_context(tc.tile_pool(name="pgemm", bufs=3, space="PSUM"))
        pout = ctx.enter_context(tc.tile_pool(name="pout", bufs=1, space="PSUM"))

        ident = const.tile([128, 128], F32)
        make_identity(nc, ident[:])
        iota_t = const.tile([128, 128], FP16, name="iota")
        nc.gpsimd.iota(
            iota_t[:],
            pattern=[[1, 128]],
            base=0,
            channel_multiplier=0,
            allow_small_or_imprecise_dtypes=True,
        )
        sidxt = const.tile([128, nidx], F32, name="sidxt")

        xr0t = const.tile([128, 4 * BS], FP16, name="xr0t")
        xr1t = [const.tile([128, 2 * BS], FP16, name=f"xr1_{s}") for s in range(4)]
        xr2t = [const.tile([128, 2 * BS], FP16, name=f"xr2_{j}") for j in range(8)]
        xr3t = [const.tile([128, 4 * BS], FP16, name=f"xr3_{j}") for j in range(8)]

        # All steady-state posts go to the sync sequencer: descriptors of
        # any one post spread across all 16 DMA engines, so a single ring
        # sustains full bandwidth; posting from Act would head-of-line
        # block the PSUM->SBUF copies behind pool-buffer semaphore waits.
        rb = {"n": 0, "startup": True}

        def ring(nbytes):
            if rb["startup"]:
                rb["n"] += 1
                return nc.scalar if rb["n"] % 2 else nc.sync
            return nc.sync

        state = {
            "xgq": [], "w0q": [], "w1q": [], "w23q": [],
            "xg_cur": None, "xg_off": 0,
            "w0_cur": None, "w0_off": 4, "w1_cur": None, "w1_off": 4,
            "w23_cur": None, "w23_off": 16,
            "pend": [], "ps": None,
        }

        def do_op(op):
            kind, arg = op
            if kind == "xr0":
                ring(4 * BS * 256).dma_start(xr0t[:], xrep_d[:, 0 : 4 * BS])
            elif kind == "xr1":
                lo = (4 + arg * 2) * BS
                ring(2 * BS * 256).dma_start(
                    xr1t[arg][:], xrep_d[:, lo : lo + 2 * BS]
                )
            elif kind == "xr2":
                lo = (12 + arg * 2) * BS
                ring(2 * BS * 256).dma_start(
                    xr2t[arg][:], xrep_d[:, lo : lo + 2 * BS]
                )
            elif kind == "xr3":
                lo = (28 + arg * 4) * BS
                ring(4 * BS * 256).dma_start(
                    xr3t[arg][:], xrep_d[:, lo : lo + 4 * BS]
                )
            elif kind == "sidx":
                ring(nidx * 512).dma_start(sidxt[:], sidx_d[:])
            elif kind == "xg":
                t = xgpool.tile([128, 4096], FP16, tag="xgt", name="xgt")
                lo = arg * 4096
                hi = min(lo + 4096, NSLOT * BS)
                ring((hi - lo) * 256).dma_start(t[:, : hi - lo], xg_d[:, lo:hi])
                state["xgq"].append(t)
            elif kind == "wt0":
                t = w0pool.tile([128, 2048], FP16, tag="w0t", name="w0t")
                lo = arg * 2048
                hi = min(lo + 2048, n0s * 512)
                ring((hi - lo) * 256).dma_start(t[:, : hi - lo], wt0_d[:, lo:hi])
                state["w0q"].append(t)
            elif kind == "wt1":
                t = w1pool.tile([128, 1024], FP16, tag="w1t", name="w1t")
                lo = arg * 1024
                hi = min(lo + 1024, n1s * 256)
                ring((hi - lo) * 256).dma_start(t[:, : hi - lo], wt1_d[:, lo:hi])
                state["w1q"].append(t)
            elif kind == "wt23":
                t = w23pool.tile([128, 2048], FP16, tag="w23t", name="w23t")
                lo = arg * 2048
                hi = min(lo + 2048, n23s * 128)
                ring((hi - lo) * 256).dma_start(t[:, : hi - lo], wt23_d[:, lo:hi])
                state["w23q"].append(t)

        outT_ps = [
            pout.tile([128, BS], F32, tag=f"pout{h}", name=f"pout{h}") for h in range(2)
        ]

        def flush_pending():
            for si, ohts, zv in state["pend"]:
                m = meta[si]
                for bk, lhs in ohts:
                    nc.tensor.matmul(
                        outT_ps[bk][:],
                        lhs,
                        zv,
                        start=m["start"][bk],
                        stop=m["stop"][bk],
                        skip_group_check=True,
                    )
            state["pend"].clear()

        def epilogue(hb):
            # scale out of the bank, transpose back INTO the drained bank
            # (no pgemm-pool churn), stage once to SBUF, then DMA out.
            outT_sb = zpool.tile([128, BS], F32, tag="outT_sb", name="outT_sb", bufs=2)
            nc.scalar.mul(outT_sb[:], outT_ps[hb][:], 0.4)
            for bh in range(4):
                nc.tensor.transpose(
                    outT_ps[hb][:, bh * 128 : (bh + 1) * 128],
                    outT_sb[:, bh * 128 : (bh + 1) * 128],
                    ident[:],
                )
            stg = zpool.tile([128, BS], F32, tag="outstg", name="outstg", bufs=2)
            nc.scalar.copy(stg[:], outT_ps[hb][:])
            for bh in range(4):
                nc.sync.dma_start(
                    out_d[bh * 128 : (bh + 1) * 128, hb * 128 : (hb + 1) * 128],
                    stg[:, bh * 128 : (bh + 1) * 128],
                )

        # ---- preamble fetches (startup-critical, ring-alternating) ----
        for _key, op in pre:
            do_op(op)
        rb["startup"] = False

        # ---- main loop over slots ----
        for si, m in enumerate(meta):
            gi = m["g"]
            s = m["s"]
            if si % 2 == 0:
                for op in fetch[si // 2]:
                    do_op(op)
                ps = pgemm.tile([128, 1024], F32, tag="pg", name="ps")
                state["ps"] = ps
            ps_half = state["ps"][:, (si % 2) * 512 : (si % 2 + 1) * 512]

            # GEMM
            if gi == 0:
                if state["w0_off"] == 4:
                    state["w0_cur"] = state["w0q"].pop(0)
                    state["w0_off"] = 0
                w = state["w0_cur"]
                o = state["w0_off"] * 512
                state["w0_off"] += 1
                for kcc in range(4):
                    nc.tensor.matmul(
                        ps_half,
                        w[:, o + kcc * 128 : o + (kcc + 1) * 128],
                        xr0t[:, kcc * BS : (kcc + 1) * BS],
                        start=(kcc == 0),
                        stop=(kcc == 3),
                    )
            elif gi == 1:
                if state["w1_off"] == 4:
                    state["w1_cur"] = state["w1q"].pop(0)
                    state["w1_off"] = 0
                w = state["w1_cur"]
                o = state["w1_off"] * 256
                state["w1_off"] += 1
                for kcc in range(2):
                    nc.tensor.matmul(
                        ps_half,
                        w[:, o + kcc * 128 : o + (kcc + 1) * 128],
                        xr1t[s][:, kcc * BS : (kcc + 1) * BS],
                        start=(kcc == 0),
                        stop=(kcc == 1),
                    )
            else:
                if state["w23_off"] == 16:
                    state["w23_cur"] = state["w23q"].pop(0)
                    state["w23_off"] = 0
                w = state["w23_cur"]
                o = state["w23_off"] * 128
                state["w23_off"] += 1
                rhs = (
                    xr2t[s // 2][:, (s % 2) * BS : (s % 2 + 1) * BS]
                    if gi == 2
                    else xr3t[s // 4][:, (s % 4) * BS : (s % 4 + 1) * BS]
                )
                nc.tensor.matmul(
                    ps_half,
                    w[:, o : o + 128],
                    rhs,
                    start=True,
                    stop=True,
                )

            if si % 2 == 1:
                thresh = 2 if si >= NSLOT - 16 else 6
                if len(state["pend"]) >= thresh:
                    flush_pending()
                pair = si // 2
                if state["xg_off"] == 4 or state["xg_cur"] is None:
                    state["xg_cur"] = state["xgq"].pop(0)
                    state["xg_off"] = 0
                xgt = state["xg_cur"]
                xo = state["xg_off"] * 1024
                state["xg_off"] += 1
                z16 = zpool.tile([128, 1024], FP16, tag="z16", name="z16")
                if pair >= NSLOT // 2 - 5:
                    # tail pairs: fused PSUM-read multiply shortens the
                    # end-of-stream copy->mult->scatter chain
                    nc.vector.tensor_mul(
                        z16[:], state["ps"][:], xgt[:, xo : xo + 1024]
                    )
                else:
                    yt = ypool.tile([128, 1024], FP16, tag="yt", name="yt")
                    if pair < 6:
                        nc.vector.tensor_copy(yt[:], state["ps"][:])
                    else:
                        nc.scalar.copy(yt[:], state["ps"][:])
                    nc.vector.tensor_mul(z16[:], yt[:], xgt[:, xo : xo + 1024])
                for j in (si - 1, si):
                    mm = meta[j]
                    ohts = []
                    for bk in mm["banks"]:
                        col = mm["icol"][bk]
                        ohg = ohpool.tile([128, 128], FP16, tag="ohg", name="ohg")
                        nc.vector.tensor_scalar(
                            ohg[:],
                            iota_t[:],
                            sidxt[:, col : col + 1],
                            None,
                            mybir.AluOpType.is_equal,
                        )
                        ohts.append((bk, ohg[:]))
                    zv = z16[:, (j % 2) * 512 : (j % 2 + 1) * 512]
                    state["pend"].append((j, ohts, zv))
                if si == ep0_after or si - 1 == ep0_after:
                    flush_pending()
                    epilogue(0)

        flush_pending()
        epilogue(1)

    nc.compile()
    return nc


_NC_CACHE = None  # (key, nc)


def _make_in_maps(x, plan, cores, xgidx):
    x = np.ascontiguousarray(np.asarray(x, dtype=np.float32))
    in_maps = []
    for c in range(NCORES):
        bc, kc = divmod(c, 2)
        xsh = x[bc * BS : (bc + 1) * BS, :]
        xr = xsh[:, xgidx]  # (512 b, 128 nu, 60 t)
        xrep = np.ascontiguousarray(
            xr.transpose(1, 2, 0).reshape(128, 60 * BS).astype(np.float16)
        )
        co = cores[kc]
        A = (xsh[:, co["iks"]].T / 4.0).astype(np.float16)  # (NSLOT*128, 512)
        xg = np.ascontiguousarray(
            A.reshape(NSLOT, 128, BS).transpose(1, 0, 2).reshape(128, NSLOT * BS)
        )
        in_maps.append(
            {
                "xrep": xrep,
                "xg": xg,
                "sidx": co["sidx"],
                "wt0": co["wt0"],
                "wt1": co["wt1"],
                "wt23": co["wt23"],
            }
        )
    return in_maps


def kernel(x, W0, W1, W2, W3, bid0, bid1, bid2, bid3, W_invperm, **_unused):
    global _NC_CACHE
    plan, cores, xgidx = _host_prep(
        W0, W1, W2, W3, bid0, bid1, bid2, bid3, W_invperm
    )
    if _NC_CACHE is None or _NC_CACHE[0] != plan["key"]:
        _NC_CACHE = (plan["key"], _build_nc(plan))
    nc = _NC_CACHE[1]

    in_maps = _make_in_maps(x, plan, cores, xgidx)
    res = run_bass_kernel_spmd(nc, in_maps, core_ids=list(range(NCORES)))
    outs = [np.asarray(res.results[c]["out"], np.float32) for c in range(NCORES)]
    out = np.concatenate(
        [outs[2 * bc] + outs[2 * bc + 1] for bc in range(NCORES // 2)], axis=0
    )
    return out.astype(np.float32)


# revision 34
# speedup vs baseline: 1.0364x; 1.0185x over previous
"""Trainium2 Bass kernel for nn_EquivariantBiLinear.

Math (per batch row b):
    Y[k, b] = sum_nu W_g[mu, nu] * x[b, bid_g[nu*r+rho]]   (k = off_g + mu*r + rho)
    out[b, o] = 0.1 * sum_i Y[W_invperm[o*256+i], b] * x[b, i]

Sharding: 4-way over batch x 2-way over k-space (8 cores; host adds the
two k-partials per batch slice). Per core: 256 chunks of 128 k-rows x
512 batch cols. Per chunk: group GEMM (fp16, N=512) -> PSUM, DVE mult
by the host-gathered xg = x[b, i_k]/4 -> z (fp16), then a one-hot fp8
scatter matmul accumulates z into the persistent output PSUM bank(s).

Key trick vs the naive layout: the scatter previously needed 2 matmuls
per chunk (its 128 target o-rows span both 128-row PSUM banks). Since
the GEMM's mu-tiling within each (group, rho)-stream is free, the host
sorts each stream's k's by target o-half so almost every chunk is
*pure* (single-bank scatter: 1 matmul). Boundary/parity chunks stay
"mixed" (2 matmuls). All weights are streamed per-chunk in the sorted
order (pure layout transform on the host). The two k-cores must share
one SPMD program, so per (stream, class) chunk counts are made even by
demoting a few pure chunks to mixed; both cores then run an identical
slot sequence with different streamed data. Scatter matmuls drop from
512 to ~308 per core.

Scheduling: slots are ordered big(g0/g1)-small(g2/g3) alternating so
every PSUM-pair step carries enough PE work to hide the PSUM->SBUF
copy; xrep is loaded as 60 per-(group,rho) blocks posted in first-use
order so the first GEMM starts ~3us in; DMA fetches are batched into
>=1KB/partition slabs to cut sequencer post overhead. The o-half-0
output bank finishes before the pure-1 phase, so its epilogue overlaps
the remaining chunks.
"""

import sys

if "/opt/trn_rl_repo" not in sys.path:
    sys.path.insert(0, "/opt/trn_rl_repo")

from contextlib import ExitStack

import numpy as np

import concourse.bacc as bacc
import concourse.mybir as mybir
import concourse.tile as tile
from concourse.bass_utils import run_bass_kernel_spmd
from concourse.masks import make_identity

GROUPS = [(512, 1, 16384), (256, 4, 4096), (128, 16, 1024), (64, 64, 256)]
OFFS = [0, 16384, 32768, 49152]
X = 256
B = 2048
NCORES = 8
BS = 512  # batch rows per core
NSLOT = 256  # chunks per core

F32 = mybir.dt.float32
FP16 = mybir.dt.float16
FP8 = mybir.dt.float8e4

# earliest slot index per group (xrep warm-up)
GSTART = [4, 0, 20, 40]


def _streams(pairs3):
    """List of (g, stream_id, k-array in canonical order)."""
    out = []
    for gi, (n, r, m) in enumerate(GROUPS):
        off = OFFS[gi]
        if gi < 3:
            for rho in range(r):
                out.append((gi, rho, off + np.arange(m) * r + rho))
        else:
            for j, (sa, sb) in enumerate(pairs3):
                out.append(
                    (
                        gi,
                        j,
                        np.concatenate(
                            [
                                off + np.arange(m) * r + sa,
                                off + np.arange(m) * r + sb,
                            ]
                        ),
                    )
                )
    return out


def _pair_g3(perm):
    """Pair g3 rho-streams so each pair's o-half-0 count sums to exactly
    256 where possible: such pairs tile into 2+2 pure chunks with no
    mixed boundary and no parity demotion."""
    H = perm // (X * X // 2)
    off = OFFS[3]
    n0 = [
        int((H[off + np.arange(256) * 64 + s] == 0).sum()) for s in range(64)
    ]
    by_val = {}
    for s, v in enumerate(n0):
        by_val.setdefault(v, []).append(s)
    pairs = []
    used = [False] * 64
    for s in range(64):
        if used[s]:
            continue
        want = 256 - n0[s]
        cand = [t for t in by_val.get(want, []) if not used[t] and t != s]
        if cand:
            t = cand[0]
            used[s] = used[t] = True
            pairs.append((s, t))
    rest = [s for s in range(64) if not used[s]]
    rest.sort(key=lambda s: n0[s])
    while rest:
        pairs.append((rest.pop(0), rest.pop(-1)))
    assert len(pairs) == 32
    return pairs


def _order_slots(per_group):
    """Merge the 4 per-group item lists into one slot order: big (g0/g1)
    and small (g2/g3) alternate so PSUM pairs carry steady PE work;
    within each size class groups interleave proportionally; GSTART
    delays a group until its xrep blocks can be resident."""

    def mk_queue(groups):
        lists = {g: list(per_group[g]) for g in groups}
        tot = {g: max(1, len(lists[g])) for g in groups}
        idx = {g: 0 for g in groups}

        def pop(slot_i, force=False):
            best, bestv = None, -1.0
            for g in groups:
                if idx[g] >= len(lists[g]):
                    continue
                if slot_i < GSTART[g] and not force:
                    continue
                v = (len(lists[g]) - idx[g]) / tot[g]
                if v > bestv:
                    best, bestv = g, v
            if best is None:
                return None
            it = lists[best][idx[best]]
            idx[best] += 1
            return it

        return pop

    nb = len(per_group[0]) + len(per_group[1])
    ns = len(per_group[2]) + len(per_group[3])
    big = mk_queue([1, 0])
    small = mk_queue([2, 3])
    out = []
    want_big = False
    for _ in range(nb + ns):
        want_big = not want_big
        it = (big if want_big else small)(len(out))
        if it is None:
            it = (small if want_big else big)(len(out))
        if it is None:
            it = big(len(out), force=True) or small(len(out), force=True)
        out.append(it)
    return out


def _plan(perm):
    """Chunk the k-space into o-half-sorted 128-tiles and build the
    shared slot schedule."""
    H = perm // (X * X // 2)
    pairs3 = _pair_g3(perm)

    chunks = {}  # (g, s, cls) -> list of klist arrays
    for gi, s, ks in _streams(pairs3):
        hh = H[ks]
        order = np.argsort(hh, kind="stable")
        ks = ks[order]
        n0 = int((hh == 0).sum())
        a0, c = divmod(n0, 128)
        mx = 1 if c else 0
        a1 = (len(ks) - n0) // 128
        guard = 0
        while (a0 % 2) or (mx % 2) or (a1 % 2):
            if a0 % 2:
                a0 -= 1
                mx += 1
            elif a1 % 2:
                a1 -= 1
                mx += 1
            guard += 1
            assert guard < 8
        tiles = [ks[t * 128 : (t + 1) * 128] for t in range(len(ks) // 128)]
        chunks[(gi, s, 0)] = tiles[:a0]
        chunks[(gi, s, 1)] = tiles[a0 : a0 + mx]
        chunks[(gi, s, 2)] = tiles[a0 + mx :]

    chunk_of = [{}, {}]
    for key, lst in chunks.items():
        assert len(lst) % 2 == 0, (key, len(lst))
        chunk_of[0][key] = lst[0::2]
        chunk_of[1][key] = lst[1::2]

    per_group = {ph: {g: [] for g in range(4)} for ph in range(2)}
    for gi, s, _ks in _streams(pairs3):
        for cls in (0, 1, 2):
            nn = len(chunk_of[0][(gi, s, cls)])
            ph = 0 if cls < 2 else 1
            per_group[ph][gi].extend([(gi, s, cls)] * nn)

    # Blend bank0-touching (cls 0/1) and pure-1 slots into one stream so
    # the DMA-heavy mix is uniform (the old A-then-B split made the A
    # phase locally DMA-bound). Front-load bank0 work to finish by ~94%
    # of the timeline so its epilogue still overlaps remaining slots.
    merged = {}
    for g in range(4):
        A, Bl = per_group[0][g], per_group[1][g]
        a, b = len(A), len(Bl)
        n = a + b
        lst = []
        ai = bi = 0
        for i in range(n):
            take_a = ai < a and (
                bi >= b or ai / max(1, a) <= i / (0.94 * max(1, n))
            )
            if take_a:
                lst.append(A[ai])
                ai += 1
            else:
                lst.append(Bl[bi])
                bi += 1
        merged[g] = lst
    slots = _order_slots(merged)
    assert len(slots) == NSLOT

    meta = []
    i0 = i1 = i23 = 0
    icol = 0
    for si, (gi, s, cls) in enumerate(slots):
        if gi == 0:
            wt = ("wt0", i0 * 512)
            i0 += 1
        elif gi == 1:
            wt = ("wt1", i1 * 256)
            i1 += 1
        else:
            wt = ("wt23", i23 * 128)
            i23 += 1
        banks = [0] if cls == 0 else ([1] if cls == 2 else [0, 1])
        cols = {}
        for bk in banks:
            cols[bk] = icol
            icol += 1
        meta.append(dict(g=gi, s=s, cls=cls, wt=wt, banks=banks, icol=cols))
    nidx = (icol + 15) // 16 * 16

    first = {0: None, 1: None}
    last = {0: None, 1: None}
    for si, m in enumerate(meta):
        for bk in m["banks"]:
            if first[bk] is None:
                first[bk] = si
            last[bk] = si
    for si, m in enumerate(meta):
        m["start"] = {bk: si == first[bk] for bk in m["banks"]}
        m["stop"] = {bk: si == last[bk] for bk in m["banks"]}
    ep0_after = last[0]

    counts = (i0, i1, i23)
    return slots, chunk_of, meta, nidx, ep0_after, counts, pairs3


def _host_prep(W0, W1, W2, W3, bid0, bid1, bid2, bid3, W_invperm):
    """Pure layout transforms of weights/indices (no arithmetic on data)."""
    import ml_dtypes

    Ws = [np.asarray(W) for W in (W0, W1, W2, W3)]
    bids = [np.asarray(b).astype(np.int64) for b in (bid0, bid1, bid2, bid3)]
    ivp = np.asarray(W_invperm).astype(np.int64)
    perm = np.empty(X * X, np.int64)
    perm[ivp] = np.arange(X * X)

    slots, chunk_of, meta, nidx, ep0_after, counts, pairs3 = _plan(perm)

    wt = [
        np.ascontiguousarray(W.reshape(m, n).T.astype(np.float16))
        for (n, r, m), W in zip(GROUPS, Ws)
    ]

    pr = np.arange(128)
    cores = []
    for kc in range(2):
        qidx = {key: 0 for key in chunk_of[kc]}
        wt0p = np.zeros((128, counts[0] * 512), np.float16)
        wt1p = np.zeros((128, counts[1] * 256), np.float16)
        wt23p = np.zeros((128, counts[2] * 128), np.float16)
        sidx = np.full((128, nidx), -1.0, np.float32)
        iks = np.empty(NSLOT * 128, np.int64)
        for si, m in enumerate(meta):
            gi, s, cls = slots[si]
            ks = chunk_of[kc][(gi, s, cls)][qidx[(gi, s, cls)]]
            qidx[(gi, s, cls)] += 1
            off = OFFS[gi]
            n, r, _m = GROUPS[gi]
            kind, woff = m["wt"]
            if gi == 3:
                mu = (ks - off) // r
                rho = (ks - off) % r
                sub = np.where(rho == pairs3[s][0], 0, 1)
                assert np.all((rho == pairs3[s][0]) | (rho == pairs3[s][1]))
                blk = np.zeros((128, 128), np.float16)
                rows = sub[None, :] * 64 + np.arange(64)[:, None]  # (64, 128)
                blk[rows, np.arange(128)[None, :]] = wt[3][:, mu]
                wt23p[:, woff : woff + 128] = blk
            else:
                mu = (ks - off) // r
                cols = wt[gi][:, mu]  # (n, 128)
                if gi == 0:
                    for kcc in range(4):
                        wt0p[:, woff + kcc * 128 : woff + (kcc + 1) * 128] = cols[
                            kcc * 128 : (kcc + 1) * 128, :
                        ]
                elif gi == 1:
                    for kcc in range(2):
                        wt1p[:, woff + kcc * 128 : woff + (kcc + 1) * 128] = cols[
                            kcc * 128 : (kcc + 1) * 128, :
                        ]
                else:
                    wt23p[:, woff : woff + 128] = cols
            pk = perm[ks]
            iks[si * 128 : (si + 1) * 128] = pk % X
            ok = pk // X
            for bk, col in m["icol"].items():
                sel = (ok // 128) == bk
                sidx[pr[sel], col] = (ok % 128)[sel]
        cores.append(dict(wt0=wt0p, wt1=wt1p, wt23=wt23p, sidx=sidx, iks=iks))

    # x-gather column map for xrep (identical to reference layout)
    colsl = []
    b0 = bids[0]
    for kcc in range(4):
        colsl.append(b0[kcc * 128 : (kcc + 1) * 128])
    b1 = bids[1].reshape(256, 4)
    for rho in range(4):
        for kcc in range(2):
            colsl.append(b1[kcc * 128 : (kcc + 1) * 128, rho])
    b2 = bids[2].reshape(128, 16)
    for rho in range(16):
        colsl.append(b2[:, rho])
    b3 = bids[3].reshape(64, 64)
    for sa, sb in pairs3:
        colsl.append(b3[pr % 64, np.where(pr // 64 == 0, sa, sb)])
    xgidx = np.ascontiguousarray(np.stack(colsl, axis=1).astype(np.int64))

    plan_key = (
        tuple(tuple(p) for p in pairs3),
        tuple(slots),
        tuple(tuple(sorted(m["icol"].items())) for m in meta),
        nidx,
        ep0_after,
        counts,
    )
    plan = dict(
        slots=slots, meta=meta, nidx=nidx, ep0_after=ep0_after, counts=counts,
        key=hash(repr(plan_key)),
    )
    return plan, cores, xgidx


def _xrep_block(gi, s, kcc=0):
    """xrep plane block index for (group, stream[, kc])."""
    if gi == 0:
        return kcc
    if gi == 1:
        return 4 + s * 2 + kcc
    if gi == 2:
        return 12 + s
    return 28 + s


def _build_nc(plan):
    meta = plan["meta"]
    nidx = plan["nidx"]
    ep0_after = plan["ep0_after"]
    n0s, n1s, n23s = plan["counts"]

    nc = bacc.Bacc("TRN2", target_bir_lowering=False, debug=False, num_devices=NCORES)

    xrep_d = nc.dram_tensor("xrep", [128, 60 * BS], FP16, kind="ExternalInput")
    xg_d = nc.dram_tensor("xg", [128, NSLOT * BS], FP16, kind="ExternalInput")
    wt0_d = nc.dram_tensor("wt0", [128, n0s * 512], FP16, kind="ExternalInput")
    wt1_d = nc.dram_tensor("wt1", [128, n1s * 256], FP16, kind="ExternalInput")
    wt23_d = nc.dram_tensor("wt23", [128, n23s * 128], FP16, kind="ExternalInput")
    sidx_d = nc.dram_tensor("sidx", [128, nidx], F32, kind="ExternalInput")
    out_d = nc.dram_tensor("out", [BS, X], F32, kind="ExternalOutput")

    # ---- fetch schedule: batched slabs, attached to the pair that is
    # PF pairs ahead of first use (negative -> preamble) ----
    PF = 6
    npair = NSLOT // 2
    fetch = [[] for _ in range(npair)]
    pre = []  # preamble fetches, in priority order

    def sched(first_use_pair, op, rank=3):
        p = first_use_pair - PF
        if p < 0:
            pre.append(((first_use_pair, rank), op))
        else:
            fetch[p].append(op)

    # xrep fetch units: g0 one [128,4BS]; g1 per-stream [128,2BS];
    # g2 stream-pairs [128,2BS]; g3 stream-quads [128,4BS]
    xrep_first = {}
    for si, m in enumerate(meta):
        gi, s = m["g"], m["s"]
        if gi == 0:
            unit = ("xr0", 0)
        elif gi == 1:
            unit = ("xr1", s)
        elif gi == 2:
            unit = ("xr2", s // 2)
        else:
            unit = ("xr3", s // 4)
        if unit not in xrep_first:
            xrep_first[unit] = (si // 2, si)
    for unit, (fu, _fs) in sorted(xrep_first.items(), key=lambda kv: kv[1]):
        sched(fu, unit, rank=1)

    # xg slabs of 4 pairs
    for slab in range((npair + 3) // 4):
        sched(slab * 4, ("xg", slab), rank=4)
    # wt windows: wt0 [4 g0-slots], wt1 [4 g1-slots], wt23 [16 slots]
    seen = set()
    for si, m in enumerate(meta):
        kind, woff = m["wt"]
        w = woff // 2048 if kind != "wt1" else woff // 1024
        if (kind, w) not in seen:
            seen.add((kind, w))
            sched(si // 2, (kind, w), rank=0 if kind == "wt1" else 2)
    sched(0, ("sidx", 0), rank=5)
    pre.sort(key=lambda kv: kv[0])

    with tile.TileContext(nc) as tc, ExitStack() as ctx:
        const = ctx.enter_context(tc.tile_pool(name="const", bufs=1))
        w0pool = ctx.enter_context(tc.tile_pool(name="w0pool", bufs=4))
        w1pool = ctx.enter_context(tc.tile_pool(name="w1pool", bufs=4))
        w23pool = ctx.enter_context(tc.tile_pool(name="w23pool", bufs=4))
        xgpool = ctx.enter_context(tc.tile_pool(name="xgpool", bufs=4))
        ohpool = ctx.enter_context(tc.tile_pool(name="ohpool", bufs=16))
        ypool = ctx.enter_context(tc.tile_pool(name="ypool", bufs=4))
        zpool = ctx.enter_context(tc.tile_pool(name="zpool", bufs=8))
        pgemm = ctx.enter_context(tc.tile_pool(name="pgemm", bufs=3, space="PSUM"))
        pout = ctx.enter_context(tc.tile_pool(name="pout", bufs=1, space="PSUM"))

        ident = const.tile([128, 128], F32)
        make_identity(nc, ident[:])
        iota_t = const.tile([128, 128], FP16, name="iota")
        nc.gpsimd.iota(
            iota_t[:],
            pattern=[[1, 128]],
            base=0,
            channel_multiplier=0,
            allow_small_or_imprecise_dtypes=True,
        )
        sidxt = const.tile([128, nidx], F32, name="sidxt")

        xr0t = const.tile([128, 4 * BS], FP16, name="xr0t")
        xr1t = [const.tile([128, 2 * BS], FP16, name=f"xr1_{s}") for s in range(4)]
        xr2t = [const.tile([128, 2 * BS], FP16, name=f"xr2_{j}") for j in range(8)]
        xr3t = [const.tile([128, 4 * BS], FP16, name=f"xr3_{j}") for j in range(8)]

        # All steady-state posts go to the sync sequencer: descriptors of
        # any one post spread across all 16 DMA engines, so a single ring
        # sustains full bandwidth; posting from Act would head-of-line
        # block the PSUM->SBUF copies behind pool-buffer semaphore waits.
        rb = {"n": 0, "startup": True}

        def ring(nbytes):
            if rb["startup"]:
                rb["n"] += 1
                return nc.scalar if rb["n"] % 2 else nc.sync
            return nc.sync

        state = {
            "xgq": [], "w0q": [], "w1q": [], "w23q": [],
            "xg_cur": None, "xg_off": 0,
            "w0_cur": None, "w0_off": 4, "w1_cur": None, "w1_off": 4,
            "w23_cur": None, "w23_off": 16,
            "pend": [], "ps": None,
        }

        def do_op(op):
            kind, arg = op
            if kind == "xr0":
                ring(4 * BS * 256).dma_start(xr0t[:], xrep_d[:, 0 : 4 * BS])
            elif kind == "xr1":
                lo = (4 + arg * 2) * BS
                ring(2 * BS * 256).dma_start(
                    xr1t[arg][:], xrep_d[:, lo : lo + 2 * BS]
                )
            elif kind == "xr2":
                lo = (12 + arg * 2) * BS
                ring(2 * BS * 256).dma_start(
                    xr2t[arg][:], xrep_d[:, lo : lo + 2 * BS]
                )
            elif kind == "xr3":
                lo = (28 + arg * 4) * BS
                ring(4 * BS * 256).dma_start(
                    xr3t[arg][:], xrep_d[:, lo : lo + 4 * BS]
                )
            elif kind == "sidx":
                ring(nidx * 512).dma_start(sidxt[:], sidx_d[:])
            elif kind == "xg":
                t = xgpool.tile([128, 4096], FP16, tag="xgt", name="xgt")
                lo = arg * 4096
                hi = min(lo + 4096, NSLOT * BS)
                ring((hi - lo) * 256).dma_start(t[:, : hi - lo], xg_d[:, lo:hi])
                state["xgq"].append(t)
            elif kind == "wt0":
                t = w0pool.tile([128, 2048], FP16, tag="w0t", name="w0t")
                lo = arg * 2048
                hi = min(lo + 2048, n0s * 512)
                ring((hi - lo) * 256).dma_start(t[:, : hi - lo], wt0_d[:, lo:hi])
                state["w0q"].append(t)
            elif kind == "wt1":
                t = w1pool.tile([128, 1024], FP16, tag="w1t", name="w1t")
                lo = arg * 1024
                hi = min(lo + 1024, n1s * 256)
                ring((hi - lo) * 256).dma_start(t[:, : hi - lo], wt1_d[:, lo:hi])
                state["w1q"].append(t)
            elif kind == "wt23":
                t = w23pool.tile([128, 2048], FP16, tag="w23t", name="w23t")
                lo = arg * 2048
                hi = min(lo + 2048, n23s * 128)
                ring((hi - lo) * 256).dma_start(t[:, : hi - lo], wt23_d[:, lo:hi])
                state["w23q"].append(t)

        outT_ps = [
            pout.tile([128, BS], F32, tag=f"pout{h}", name=f"pout{h}") for h in range(2)
        ]

        def flush_pending():
            for si, ohts, zv in state["pend"]:
                m = meta[si]
                for bk, lhs in ohts:
                    nc.tensor.matmul(
                        outT_ps[bk][:],
                        lhs,
                        zv,
                        start=m["start"][bk],
                        stop=m["stop"][bk],
                        skip_group_check=True,
                    )
            state["pend"].clear()

        def epilogue(hb):
            # scale out of the bank, transpose back INTO the drained bank
            # (no pgemm-pool churn), stage once to SBUF, then DMA out.
            outT_sb = zpool.tile([128, BS], F32, tag="outT_sb", name="outT_sb", bufs=2)
            nc.scalar.mul(outT_sb[:], outT_ps[hb][:], 0.4)
            for bh in range(4):
                nc.tensor.transpose(
                    outT_ps[hb][:, bh * 128 : (bh + 1) * 128],
                    outT_sb[:, bh * 128 : (bh + 1) * 128],
                    ident[:],
                )
            stg = zpool.tile([128, BS], F32, tag="outstg", name="outstg", bufs=2)
            nc.scalar.copy(stg[:], outT_ps[hb][:])
            for bh in range(4):
                nc.sync.dma_start(
                    out_d[bh * 128 : (bh + 1) * 128, hb * 128 : (hb + 1) * 128],
                    stg[:, bh * 128 : (bh + 1) * 128],
                )

        # ---- preamble fetches (startup-critical, ring-alternating) ----
        for _key, op in pre:
            do_op(op)
        rb["startup"] = False

        # ---- main loop over slots ----
        for si, m in enumerate(meta):
            gi = m["g"]
            s = m["s"]
            if si % 2 == 0:
                for op in fetch[si // 2]:
                    do_op(op)
                ps = pgemm.tile([128, 1024], F32, tag="pg", name="ps")
                state["ps"] = ps
            ps_half = state["ps"][:, (si % 2) * 512 : (si % 2 + 1) * 512]

            # GEMM
            if gi == 0:
                if state["w0_off"] == 4:
                    state["w0_cur"] = state["w0q"].pop(0)
                    state["w0_off"] = 0
                w = state["w0_cur"]
                o = state["w0_off"] * 512
                state["w0_off"] += 1
                for kcc in range(4):
                    nc.tensor.matmul(
                        ps_half,
                        w[:, o + kcc * 128 : o + (kcc + 1) * 128],
                        xr0t[:, kcc * BS : (kcc + 1) * BS],
                        start=(kcc == 0),
                        stop=(kcc == 3),
                    )
            elif gi == 1:
                if state["w1_off"] == 4:
                    state["w1_cur"] = state["w1q"].pop(0)
                    state["w1_off"] = 0
                w = state["w1_cur"]
                o = state["w1_off"] * 256
                state["w1_off"] += 1
                for kcc in range(2):
                    nc.tensor.matmul(
                        ps_half,
                        w[:, o + kcc * 128 : o + (kcc + 1) * 128],
                        xr1t[s][:, kcc * BS : (kcc + 1) * BS],
                        start=(kcc == 0),
                        stop=(kcc == 1),
                    )
            else:
                if state["w23_off"] == 16:
                    state["w23_cur"] = state["w23q"].pop(0)
                    state["w23_off"] = 0
                w = state["w23_cur"]
                o = state["w23_off"] * 128
                state["w23_off"] += 1
                rhs = (
                    xr2t[s // 2][:, (s % 2) * BS : (s % 2 + 1) * BS]
                    if gi == 2
                    else xr3t[s // 4][:, (s % 4) * BS : (s % 4 + 1) * BS]
                )
                nc.tensor.matmul(
                    ps_half,
                    w[:, o : o + 128],
                    rhs,
                    start=True,
                    stop=True,
                )

            if si % 2 == 1:
                thresh = 2 if si >= NSLOT - 16 else 6
                if len(state["pend"]) >= thresh:
                    flush_pending()
                pair = si // 2
                if state["xg_off"] == 4 or state["xg_cur"] is None:
                    state["xg_cur"] = state["xgq"].pop(0)
                    state["xg_off"] = 0
                xgt = state["xg_cur"]
                xo = state["xg_off"] * 1024
                state["xg_off"] += 1
                z16 = zpool.tile([128, 1024], FP16, tag="z16", name="z16")
                if pair >= NSLOT // 2 - 5:
                    # tail pairs: fused PSUM-read multiply shortens the
                    # end-of-stream copy->mult->scatter chain
                    nc.vector.tensor_mul(
                        z16[:], state["ps"][:], xgt[:, xo : xo + 1024]
                    )
                else:
                    yt = ypool.tile([128, 1024], FP16, tag="yt", name="yt")
                    nc.scalar.copy(yt[:], state["ps"][:])
                    nc.vector.tensor_mul(z16[:], yt[:], xgt[:, xo : xo + 1024])
                for j in (si - 1, si):
                    mm = meta[j]
                    ohts = []
                    for bk in mm["banks"]:
                        col = mm["icol"][bk]
                        ohg = ohpool.tile([128, 128], FP16, tag="ohg", name="ohg")
                        nc.vector.tensor_scalar(
                            ohg[:],
                            iota_t[:],
                            sidxt[:, col : col + 1],
                            None,
                            mybir.AluOpType.is_equal,
                        )
                        ohts.append((bk, ohg[:]))
                    zv = z16[:, (j % 2) * 512 : (j % 2 + 1) * 512]
                    state["pend"].append((j, ohts, zv))
                if si == ep0_after or si - 1 == ep0_after:
                    flush_pending()
                    epilogue(0)

        flush_pending()
        epilogue(1)

    nc.compile()
    return nc


_NC_CACHE = None  # (key, nc)


def _make_in_maps(x, plan, cores, xgidx):
    x = np.ascontiguousarray(np.asarray(x, dtype=np.float32))
    in_maps = []
    for c in range(NCORES):
        bc, kc = divmod(c, 2)
        xsh = x[bc * BS : (bc + 1) * BS, :]
        xr = xsh[:, xgidx]  # (512 b, 128 nu, 60 t)
        xrep = np.ascontiguousarray(
            xr.transpose(1, 2, 0).reshape(128, 60 * BS).astype(np.float16)
        )
        co = cores[kc]
        A = (xsh[:, co["iks"]].T / 4.0).astype(np.float16)  # (NSLOT*128, 512)
        xg = np.ascontiguousarray(
            A.reshape(NSLOT, 128, BS).transpose(1, 0, 2).reshape(128, NSLOT * BS)
        )
        in_maps.append(
            {
                "xrep": xrep,
                "xg": xg,
                "sidx": co["sidx"],
                "wt0": co["wt0"],
                "wt1": co["wt1"],
                "wt23": co["wt23"],
            }
        )
    return in_maps


def kernel(x, W0, W1, W2, W3, bid0, bid1, bid2, bid3, W_invperm, **_unused):
    global _NC_CACHE
    plan, cores, xgidx = _host_prep(
        W0, W1, W2, W3, bid0, bid1, bid2, bid3, W_invperm
    )
    if _NC_CACHE is None or _NC_CACHE[0] != plan["key"]:
        _NC_CACHE = (plan["key"], _build_nc(plan))
    nc = _NC_CACHE[1]

    in_maps = _make_in_maps(x, plan, cores, xgidx)
    res = run_bass_kernel_spmd(nc, in_maps, core_ids=list(range(NCORES)))
    outs = [np.asarray(res.results[c]["out"], np.float32) for c in range(NCORES)]
    out = np.concatenate(
        [outs[2 * bc] + outs[2 * bc + 1] for bc in range(NCORES // 2)], axis=0
    )
    return out.astype(np.float32)
